# revision 3
# baseline (speedup 1.0000x reference)
"""MultiHeadAttention (B=2, T=2048, D=512, H=8, causal) on 8 trn2 NeuronCores.

Sharding: batch*heads across cores. Core c handles batch c//4 and heads
{2*(c%4), 2*(c%4)+1}. Each core projects Q/K/V for its two heads (weight
slices replicated), runs softmax attention with scores materialized
transposed ([keys, queries] so the softmax reduction lands on the PSUM/matmul
path instead of cross-partition ops), applies its slice of the output
projection, and writes a [T, D] partial. Host sums the 8 partials (+ output
bias) into the full [B, T, D] result.

Numerics: all large matmuls run in float32r (TF32-like, ~1.2e-4 relative),
exact-fp32 PE transposes for V, softmax without max-subtraction (scores are
O(1) by construction: Q/K projections of unit-variance data through
U(-1/sqrt(D)) weights; exp stays far from fp32 range).
"""

import numpy as np

import concourse.bass as bass
import concourse.mybir as mybir
import concourse.tile as tile
from concourse.bass_utils import run_bass_kernel_spmd

D_MODEL = 512
N_HEADS = 8
D_K = 64
B = 2
T = 2048
N_CORES = 8
P = 128
QC = 4           # query chunks of 512
QW = T // QC     # 512 queries per chunk
KT = T // P      # 16 key tiles of 128
F32 = mybir.dt.float32
F32R = mybir.dt.float32r

_BUILD_CACHE = {}


def _hoist_excess_waits(nc, max_waits=1):
    """walrus codegen supports at most one sync-wait slot per hardware
    instruction, but Tile's sem-assignment can attach several (e.g. inputs
    arriving via two HW-DGE queues). Move the excess onto same-engine no-ops
    placed just before the instruction."""
    n_fixed = 0
    for fn in nc.m.functions:
        for bb in fn.blocks:
            insts = bb.instructions
            new_list = []
            for ins in insts:
                si = ins.sync_info
                ow = list(si.on_wait or []) if si else []
                if len(ow) > max_waits and ins.is_executable():
                    for j, w in enumerate(ow[max_waits:]):
                        nop = mybir.InstNoOp(
                            name=f"waitnop{j}_{ins.name}", ins=[], outs=[]
                        )
                        nop.engine = ins.engine
                        nop.sync_info = mybir.SyncInfo(on_wait=[w], on_update=[])
                        new_list.append(nop)
                    si.on_wait = ow[:max_waits]
                    ins.sync_info = si
                    n_fixed += 1
                new_list.append(ins)
            insts[:] = new_list
    return n_fixed


def _build_nc(plan):
    """plan: 'causal' or 'full' -> one SPMD program for all 8 cores."""
    nc = bass.Bass(target_bir_lowering=False)

    qT = nc.dram_tensor("qT", [D_MODEL, T], F32, kind="ExternalInput")
    kT = nc.dram_tensor("kT", [D_MODEL, T], F32, kind="ExternalInput")
    vT = nc.dram_tensor("vT", [D_MODEL, T], F32, kind="ExternalInput")
    wqT = nc.dram_tensor("wqT", [D_MODEL, P], F32, kind="ExternalInput")
    wkT = nc.dram_tensor("wkT", [D_MODEL, P], F32, kind="ExternalInput")
    wvT = nc.dram_tensor("wvT", [D_MODEL, P], F32, kind="ExternalInput")
    bq2 = nc.dram_tensor("bq2", [P, 1], F32, kind="ExternalInput")
    bk2 = nc.dram_tensor("bk2", [P, 1], F32, kind="ExternalInput")
    bv2 = nc.dram_tensor("bv2", [P, 1], F32, kind="ExternalInput")
    woT0 = nc.dram_tensor("woT0", [D_K, D_MODEL], F32, kind="ExternalInput")
    woT1 = nc.dram_tensor("woT1", [D_K, D_MODEL], F32, kind="ExternalInput")
    idstack = nc.dram_tensor("idstack", [P, D_K], F32, kind="ExternalInput")
    ones_in = nc.dram_tensor("ones_in", [P, D_K], F32, kind="ExternalInput")
    outp = nc.dram_tensor("outp", [T, D_MODEL], F32, kind="ExternalOutput")

    def kts_of(qc):
        return list(range(KT if plan == "full" else 4 * (qc + 1)))

    def is_partial(qc, kt):
        return plan == "causal" and 4 * qc <= kt <= 4 * qc + 3

    with tile.TileContext(nc) as tc:
        with (
            tc.tile_pool(name="consts", bufs=1) as consts,
            tc.tile_pool(name="xin", bufs=3) as xin,
            tc.tile_pool(name="projT", bufs=1) as projT,
            tc.tile_pool(name="epool", bufs=4) as epool,
            tc.tile_pool(name="rpool", bufs=2) as rpool,
            tc.tile_pool(name="opool", bufs=3) as opool,
            tc.tile_pool(name="mm_ps", bufs=4, space="PSUM") as mm_ps,
            tc.tile_pool(name="ctx_ps", bufs=2, space="PSUM") as ctx_ps_pool,
        ):
            # ---- constants ----
            wq_sb = consts.tile([P, 4, P], F32R, tag="wq")
            wk_sb = consts.tile([P, 4, P], F32R, tag="wk")
            wv_sb = consts.tile([P, 4, P], F32R, tag="wv")
            nc.sync.dma_start(wq_sb, wqT.ap().bitcast(F32R).rearrange("(c p) m -> p c m", p=P))
            nc.sync.dma_start(wk_sb, wkT.ap().bitcast(F32R).rearrange("(c p) m -> p c m", p=P))
            nc.sync.dma_start(wv_sb, wvT.ap().bitcast(F32R).rearrange("(c p) m -> p c m", p=P))
            bq_sb = consts.tile([P, 1], F32, tag="bq")
            bk_sb = consts.tile([P, 1], F32, tag="bk")
            bv_sb = consts.tile([P, 1], F32, tag="bv")
            nc.sync.dma_start(bq_sb, bq2[:, :])
            nc.sync.dma_start(bk_sb, bk2[:, :])
            nc.sync.dma_start(bv_sb, bv2[:, :])
            wo0_sb = consts.tile([D_K, D_MODEL], F32R, tag="wo0")
            wo1_sb = consts.tile([D_K, D_MODEL], F32R, tag="wo1")
            nc.sync.dma_start(wo0_sb, woT0.ap().bitcast(F32R))
            nc.sync.dma_start(wo1_sb, woT1.ap().bitcast(F32R))
            ident = consts.tile([P, D_K], F32, tag="ident")
            nc.sync.dma_start(ident, idstack[:, :])
            ones_r = consts.tile([P, D_K], F32R, tag="ones")
            nc.sync.dma_start(ones_r, ones_in.ap().bitcast(F32R))

            # ---- projections ----
            qhT = projT.tile([P, T], F32R, tag="qhT")   # [2*dk, T]
            khT = projT.tile([P, T], F32R, tag="khT")
            vhT = projT.tile([P, T], F32, tag="vhT")    # fp32: transposed exactly

            def project(xT_dram, w_sb, b_sb, dst, dst_dtype_is_f32r):
                for qc in range(QC):
                    x_sb = xin.tile([P, 4, QW], F32R, tag="xin")
                    nc.sync.dma_start(
                        x_sb,
                        xT_dram.ap()
                        .bitcast(F32R)
                        .rearrange("(c p) t -> p c t", p=P)[:, :, qc * QW:(qc + 1) * QW],
                    )
                    ps = mm_ps.tile([P, QW], F32, tag="mm")
                    for kc in range(4):
                        nc.tensor.matmul(
                            ps, w_sb[:, kc, :], x_sb[:, kc, :],
                            start=(kc == 0), stop=(kc == 3),
                        )
                    nc.vector.tensor_scalar(
                        out=dst[:, qc * QW:(qc + 1) * QW],
                        in0=ps, scalar1=b_sb, scalar2=None,
                        op0=mybir.AluOpType.add,
                    )

            project(kT, wk_sb, bk_sb, khT, True)
            project(vT, wv_sb, bv_sb, vhT, False)
            project(qT, wq_sb, bq_sb, qhT, True)

            # ---- vh_aug: [Tk-part, kt, head, dk+1], last col = 1 ----
            vaug = projT.tile([P, KT, 2, D_K + 1], F32R, tag="vaug")
            for kt in range(KT):
                for h in range(2):
                    tr = mm_ps.tile([P, D_K], F32, tag="mm")
                    nc.tensor.transpose(
                        tr,
                        vhT[h * D_K:(h + 1) * D_K, kt * P:(kt + 1) * P],
                        ident[h * D_K:(h + 1) * D_K, :],
                    )
                    nc.vector.tensor_copy(vaug[:, kt, h, 0:D_K], tr)
                    nc.vector.tensor_copy(vaug[:, kt, h, D_K:D_K + 1], ones_r[:, 0:1])

            # ---- attention + output projection ----
            ctxT0 = projT.tile([D_K, T], F32R, tag="ctxT0")
            ctxT1 = projT.tile([D_K, T], F32R, tag="ctxT1")

            for qc in range(QC):
                kts = kts_of(qc)
                ctx_ps = [
                    ctx_ps_pool.tile([D_K + 1, QW], F32, tag="ctx", name=f"ctx_{qc}_{h}")
                    for h in range(2)
                ]
                for kt in kts:
                    for h in range(2):
                        hs = slice(h * D_K, (h + 1) * D_K)
                        s_ps = mm_ps.tile([P, QW], F32, tag="mm")
                        nc.tensor.matmul(
                            s_ps,
                            khT[hs, kt * P:(kt + 1) * P],
                            qhT[hs, qc * QW:(qc + 1) * QW],
                            start=True, stop=True,
                        )
                        e_sb = epool.tile([P, QW], F32R, tag="e")
                        nc.scalar.activation(
                            e_sb, s_ps, mybir.ActivationFunctionType.Exp
                        )
                        if is_partial(qc, kt):
                            nc.gpsimd.affine_select(
                                out=e_sb, in_=e_sb,
                                compare_op=mybir.AluOpType.is_ge,
                                fill=0.0,
                                base=qc * QW - kt * P,
                                pattern=[[1, QW]],
                                channel_multiplier=-1,
                            )
                        nc.tensor.matmul(
                            ctx_ps[h],
                            vaug[:, kt, h, :],
                            e_sb,
                            start=(kt == kts[0]), stop=(kt == kts[-1]),
                        )
                for h, ctxT in ((0, ctxT0), (1, ctxT1)):
                    recip = rpool.tile([D_K + 1, QW], F32R, tag="recip")
                    with nc.allow_low_precision(reason="f32r softmax denom"):
                        nc.vector.reciprocal(
                            out=recip[D_K:D_K + 1, :], in_=ctx_ps[h][D_K:D_K + 1, :]
                        )
                    bc_ps = mm_ps.tile([D_K, QW], F32, tag="mm")
                    nc.tensor.matmul(
                        bc_ps,
                        ones_r[D_K:D_K + 1, :],
                        recip[D_K:D_K + 1, :],
                        start=True, stop=True,
                    )
                    bcast = rpool.tile([D_K, QW], F32R, tag="bcast")
                    nc.vector.tensor_copy(bcast, bc_ps)
                    nc.vector.tensor_tensor(
                        ctxT[:, qc * QW:(qc + 1) * QW],
                        ctx_ps[h][0:D_K, :],
                        bcast,
                        mybir.AluOpType.mult,
                    )

                # O-projection for this query chunk (4 token tiles of 128)
                for qt in range(qc * 4, (qc + 1) * 4):
                    o_ps = mm_ps.tile([P, D_MODEL], F32, tag="mm")
                    nc.tensor.matmul(
                        o_ps, ctxT0[:, qt * P:(qt + 1) * P], wo0_sb,
                        start=True, stop=False,
                    )
                    nc.tensor.matmul(
                        o_ps, ctxT1[:, qt * P:(qt + 1) * P], wo1_sb,
                        start=False, stop=True,
                    )
                    o_sb = opool.tile([P, D_MODEL], F32, tag="o")
                    nc.vector.tensor_copy(o_sb, o_ps)
                    nc.sync.dma_start(outp[qt * P:(qt + 1) * P, :], o_sb)

    _hoist_excess_waits(nc)
    return nc


def get_nc(plan):
    if plan not in _BUILD_CACHE:
        _BUILD_CACHE[plan] = _build_nc(plan)
    return _BUILD_CACHE[plan]


def make_in_maps(q, k, v, wq, bq, wk, bk, wv, bv, wo):
    scale = 1.0 / np.sqrt(D_K)
    idstack = np.concatenate([np.eye(D_K, dtype=np.float32)] * 2, axis=0)
    ones_in = np.ones((P, D_K), dtype=np.float32)
    in_maps = []
    for c in range(N_CORES):
        b = c // (N_CORES // B)
        h0 = 2 * (c % (N_CORES // B))
        ds = slice(h0 * D_K, (h0 + 2) * D_K)
        in_maps.append({
            "qT": np.ascontiguousarray(q[b].T),
            "kT": np.ascontiguousarray(k[b].T),
            "vT": np.ascontiguousarray(v[b].T),
            "wqT": np.ascontiguousarray((wq[ds] * scale).T),
            "wkT": np.ascontiguousarray(wk[ds].T),
            "wvT": np.ascontiguousarray(wv[ds].T),
            "bq2": (bq[ds] * scale).reshape(P, 1).astype(np.float32),
            "bk2": bk[ds].reshape(P, 1).astype(np.float32),
            "bv2": bv[ds].reshape(P, 1).astype(np.float32),
            "woT0": np.ascontiguousarray(wo[:, h0 * D_K:(h0 + 1) * D_K].T),
            "woT1": np.ascontiguousarray(wo[:, (h0 + 1) * D_K:(h0 + 2) * D_K].T),
            "idstack": idstack,
            "ones_in": ones_in,
        })
    return in_maps


def classify_mask(mask):
    m = np.asarray(mask)
    if m.all():
        return "full"
    tril = np.tril(np.ones((T, T), dtype=bool))
    if all(np.array_equal(m[b, 0], tril) for b in range(m.shape[0])):
        return "causal"
    return "general"


def _numpy_reference(q, k, v, mask, wq, bq, wk, bk, wv, bv, wo, bo):
    """Fallback for mask patterns the device program doesn't cover."""
    qh = (q @ wq.T + bq).reshape(B, T, N_HEADS, D_K).transpose(0, 2, 1, 3)
    kh = (k @ wk.T + bk).reshape(B, T, N_HEADS, D_K).transpose(0, 2, 1, 3)
    vh = (v @ wv.T + bv).reshape(B, T, N_HEADS, D_K).transpose(0, 2, 1, 3)
    s = np.einsum("bhqd,bhkd->bhqk", qh, kh) / np.sqrt(D_K).astype(np.float32)
    s = np.where(mask, s, -np.inf)
    all_masked = ~mask.any(axis=-1, keepdims=True)
    s = np.where(all_masked, 0.0, s)
    s = s - s.max(axis=-1, keepdims=True)
    e = np.exp(s)
    p = e / e.sum(axis=-1, keepdims=True)
    ctx = np.einsum("bhqk,bhkd->bhqd", p, vh)
    ctx = ctx.transpose(0, 2, 1, 3).reshape(B, T, D_MODEL)
    return (ctx @ wo.T + bo).astype(np.float32)


def kernel(q, k, v, mask, wq, bq, wk, bk, wv, bv, wo, bo, _trace=False):
    q, k, v = (np.asarray(x, dtype=np.float32) for x in (q, k, v))
    mask = np.asarray(mask, dtype=bool)
    wq, bq, wk, bk, wv, bv, wo, bo = (
        np.asarray(x, dtype=np.float32) for x in (wq, bq, wk, bk, wv, bv, wo, bo)
    )

    plan = classify_mask(mask)
    if plan == "general":
        return _numpy_reference(q, k, v, mask, wq, bq, wk, bk, wv, bv, wo, bo)

    nc = get_nc(plan)
    in_maps = make_in_maps(q, k, v, wq, bq, wk, bk, wv, bv, wo)
    res = run_bass_kernel_spmd(
        nc, in_maps, core_ids=list(range(N_CORES)), trace=_trace
    )

    out = np.zeros((B, T, D_MODEL), dtype=np.float32)
    for c in range(N_CORES):
        out[c // (N_CORES // B)] += res.results[c]["outp"]
    out += bo[None, None, :]
    if _trace:
        kernel.last_exec_time_ns = res.exec_time_ns
    return out


# revision 5
# speedup vs baseline: 1.0986x; 1.0986x over previous
"""MultiHeadAttention (B=2, T=2048, D=512, H=8, causal) on 8 trn2 NeuronCores.

Sharding: batch*heads across cores. Core c handles batch c//4 and heads
{2*(c%4), 2*(c%4)+1}. Each core projects Q/K/V for its two heads (weight
slices replicated), runs softmax attention with scores materialized
transposed ([keys, queries] so the softmax reduction lands on the PSUM/matmul
path instead of cross-partition ops), applies its slice of the output
projection, and writes a [T, D] partial. Host sums the 8 partials (+ output
bias) into the full [B, T, D] result.

Numerics: all large matmuls run in float32r (TF32-like, ~1.2e-4 relative),
exact-fp32 PE transposes for V, softmax without max-subtraction (scores are
O(1) by construction: Q/K projections of unit-variance data through
U(-1/sqrt(D)) weights; exp stays far from fp32 range).
"""

import numpy as np

import concourse.bass as bass
import concourse.mybir as mybir
import concourse.tile as tile
from concourse.bass_utils import run_bass_kernel_spmd

D_MODEL = 512
N_HEADS = 8
D_K = 64
B = 2
T = 2048
N_CORES = 8
P = 128
QC = 4           # query chunks of 512
QW = T // QC     # 512 queries per chunk
KT = T // P      # 16 key tiles of 128
F32 = mybir.dt.float32
F32R = mybir.dt.float32r

_BUILD_CACHE = {}


def _hoist_excess_waits(nc, max_waits=1):
    """walrus codegen supports at most one sync-wait slot per hardware
    instruction, but Tile's sem-assignment can attach several (e.g. inputs
    arriving via two HW-DGE queues). Move the excess onto same-engine no-ops
    placed just before the instruction."""
    n_fixed = 0
    for fn in nc.m.functions:
        for bb in fn.blocks:
            insts = bb.instructions
            new_list = []
            for ins in insts:
                si = ins.sync_info
                ow = list(si.on_wait or []) if si else []
                if len(ow) > max_waits and ins.is_executable():
                    for j, w in enumerate(ow[max_waits:]):
                        nop = mybir.InstNoOp(
                            name=f"waitnop{j}_{ins.name}", ins=[], outs=[]
                        )
                        nop.engine = ins.engine
                        nop.sync_info = mybir.SyncInfo(on_wait=[w], on_update=[])
                        new_list.append(nop)
                    si.on_wait = ow[:max_waits]
                    ins.sync_info = si
                    n_fixed += 1
                new_list.append(ins)
            insts[:] = new_list
    return n_fixed


def _build_nc(plan):
    """plan: 'causal' or 'full' -> one SPMD program for all 8 cores."""
    nc = bass.Bass(target_bir_lowering=False)

    qT = nc.dram_tensor("qT", [D_MODEL, T], F32, kind="ExternalInput")
    kT = nc.dram_tensor("kT", [D_MODEL, T], F32, kind="ExternalInput")
    vT = nc.dram_tensor("vT", [D_MODEL, T], F32, kind="ExternalInput")
    wqT = nc.dram_tensor("wqT", [D_MODEL, P], F32, kind="ExternalInput")
    wkT = nc.dram_tensor("wkT", [D_MODEL, P], F32, kind="ExternalInput")
    wvT = nc.dram_tensor("wvT", [D_MODEL, P], F32, kind="ExternalInput")
    bq2 = nc.dram_tensor("bq2", [P, 1], F32, kind="ExternalInput")
    bk2 = nc.dram_tensor("bk2", [P, 1], F32, kind="ExternalInput")
    bv2 = nc.dram_tensor("bv2", [P, 1], F32, kind="ExternalInput")
    woT0 = nc.dram_tensor("woT0", [D_K, D_MODEL], F32, kind="ExternalInput")
    woT1 = nc.dram_tensor("woT1", [D_K, D_MODEL], F32, kind="ExternalInput")
    idstack = nc.dram_tensor("idstack", [P, D_K], F32, kind="ExternalInput")
    ones_in = nc.dram_tensor("ones_in", [P, D_K], F32, kind="ExternalInput")
    outp = nc.dram_tensor("outp", [T, D_MODEL], F32, kind="ExternalOutput")

    def kts_of(qc):
        return list(range(KT if plan == "full" else 4 * (qc + 1)))

    def is_partial(qc, kt):
        return plan == "causal" and 4 * qc <= kt <= 4 * qc + 3

    with tile.TileContext(nc) as tc:
        with (
            tc.tile_pool(name="consts", bufs=1) as consts,
            tc.tile_pool(name="xin", bufs=4) as xin,
            tc.tile_pool(name="projT", bufs=1) as projT,
            tc.tile_pool(name="epool", bufs=6) as epool,
            tc.tile_pool(name="rpool", bufs=2) as rpool,
            tc.tile_pool(name="opool", bufs=3) as opool,
            tc.tile_pool(name="mm_ps", bufs=4, space="PSUM") as mm_ps,
            tc.tile_pool(name="ctx_ps", bufs=2, space="PSUM") as ctx_ps_pool,
        ):
            # ---- constants ----
            wq_sb = consts.tile([P, 4, P], F32R, tag="wq")
            wk_sb = consts.tile([P, 4, P], F32R, tag="wk")
            wv_sb = consts.tile([P, 4, P], F32R, tag="wv")
            nc.sync.dma_start(wq_sb, wqT.ap().bitcast(F32R).rearrange("(c p) m -> p c m", p=P))
            nc.sync.dma_start(wk_sb, wkT.ap().bitcast(F32R).rearrange("(c p) m -> p c m", p=P))
            nc.sync.dma_start(wv_sb, wvT.ap().bitcast(F32R).rearrange("(c p) m -> p c m", p=P))
            bq_sb = consts.tile([P, 1], F32, tag="bq")
            bk_sb = consts.tile([P, 1], F32, tag="bk")
            bv_sb = consts.tile([P, 1], F32, tag="bv")
            nc.sync.dma_start(bq_sb, bq2[:, :])
            nc.sync.dma_start(bk_sb, bk2[:, :])
            nc.sync.dma_start(bv_sb, bv2[:, :])
            wo0_sb = consts.tile([D_K, D_MODEL], F32R, tag="wo0")
            wo1_sb = consts.tile([D_K, D_MODEL], F32R, tag="wo1")
            nc.sync.dma_start(wo0_sb, woT0.ap().bitcast(F32R))
            nc.sync.dma_start(wo1_sb, woT1.ap().bitcast(F32R))
            ident = consts.tile([P, D_K], F32, tag="ident")
            nc.sync.dma_start(ident, idstack[:, :])
            ones_r = consts.tile([P, D_K], F32R, tag="ones")
            nc.sync.dma_start(ones_r, ones_in.ap().bitcast(F32R))

            # ---- projections ----
            qhT = projT.tile([P, T], F32R, tag="qhT")   # [2*dk, T]
            khT = projT.tile([P, T], F32R, tag="khT")
            vhT = projT.tile([P, T], F32, tag="vhT")    # fp32: transposed exactly

            vaug = projT.tile([P, KT, 2, D_K + 1], F32R, tag="vaug")

            def project_chunk(xT_dram, w_sb, b_sb, dst, qc, xname):
                x_sb = xin.tile([P, 4, QW], F32R, tag="xin", name=f"x_{xname}_{qc}")
                nc.sync.dma_start(
                    x_sb,
                    xT_dram.ap()
                    .bitcast(F32R)
                    .rearrange("(c p) t -> p c t", p=P)[:, :, qc * QW:(qc + 1) * QW],
                )
                ps = mm_ps.tile([P, QW], F32, tag="mm", name=f"ps_{xname}_{qc}")
                for kc in range(4):
                    nc.tensor.matmul(
                        ps, w_sb[:, kc, :], x_sb[:, kc, :],
                        start=(kc == 0), stop=(kc == 3),
                    )
                nc.vector.tensor_scalar(
                    out=dst[:, qc * QW:(qc + 1) * QW],
                    in0=ps, scalar1=b_sb, scalar2=None,
                    op0=mybir.AluOpType.add,
                )

            def vaug_chunk(qc):
                # transpose v-chunk qc's 4 key tiles into vaug (exact fp32)
                for kt in range(4 * qc, 4 * qc + 4):
                    for h in range(2):
                        tr = mm_ps.tile([P, D_K], F32, tag="mm", name=f"tr_{kt}_{h}")
                        nc.tensor.transpose(
                            tr,
                            vhT[h * D_K:(h + 1) * D_K, kt * P:(kt + 1) * P],
                            ident[h * D_K:(h + 1) * D_K, :],
                        )
                        nc.vector.tensor_copy(vaug[:, kt, h, 0:D_K], tr)
                        nc.vector.tensor_copy(vaug[:, kt, h, D_K:D_K + 1], ones_r[:, 0:1])

            # chunk order: unblock attention qc=0 (needs k0,v0,q0) asap
            project_chunk(kT, wk_sb, bk_sb, khT, 0, "k")
            project_chunk(qT, wq_sb, bq_sb, qhT, 0, "q")
            project_chunk(vT, wv_sb, bv_sb, vhT, 0, "v")
            vaug_chunk(0)
            for qc in range(1, QC):
                project_chunk(kT, wk_sb, bk_sb, khT, qc, "k")
                project_chunk(vT, wv_sb, bv_sb, vhT, qc, "v")
                project_chunk(qT, wq_sb, bq_sb, qhT, qc, "q")
                vaug_chunk(qc)

            # ---- attention + output projection ----
            ctxT0 = projT.tile([D_K, T], F32R, tag="ctxT0")
            ctxT1 = projT.tile([D_K, T], F32R, tag="ctxT1")

            for qc in range(QC):
                kts = kts_of(qc)
                ctx_ps = [
                    ctx_ps_pool.tile([D_K + 1, QW], F32, tag="ctx", name=f"ctx_{qc}_{h}")
                    for h in range(2)
                ]
                for kt in kts:
                    for h in range(2):
                        hs = slice(h * D_K, (h + 1) * D_K)
                        s_ps = mm_ps.tile([P, QW], F32, tag="mm")
                        nc.tensor.matmul(
                            s_ps,
                            khT[hs, kt * P:(kt + 1) * P],
                            qhT[hs, qc * QW:(qc + 1) * QW],
                            start=True, stop=True,
                        )
                        e_sb = epool.tile([P, QW], F32R, tag="e")
                        nc.scalar.activation(
                            e_sb, s_ps, mybir.ActivationFunctionType.Exp
                        )
                        if is_partial(qc, kt):
                            nc.gpsimd.affine_select(
                                out=e_sb, in_=e_sb,
                                compare_op=mybir.AluOpType.is_ge,
                                fill=0.0,
                                base=qc * QW - kt * P,
                                pattern=[[1, QW]],
                                channel_multiplier=-1,
                            )
                        nc.tensor.matmul(
                            ctx_ps[h],
                            vaug[:, kt, h, :],
                            e_sb,
                            start=(kt == kts[0]), stop=(kt == kts[-1]),
                        )
                for h, ctxT in ((0, ctxT0), (1, ctxT1)):
                    recip = rpool.tile([D_K + 1, QW], F32R, tag="recip")
                    lnd = rpool.tile([D_K + 1, QW], F32, tag="lnd")
                    nc.scalar.activation(
                        lnd[D_K:D_K + 1, :], ctx_ps[h][D_K:D_K + 1, :],
                        mybir.ActivationFunctionType.Ln,
                    )
                    nc.scalar.activation(
                        recip[D_K:D_K + 1, :], lnd[D_K:D_K + 1, :],
                        mybir.ActivationFunctionType.Exp, scale=-1.0,
                    )
                    bc_ps = mm_ps.tile([D_K, QW], F32, tag="mm")
                    nc.tensor.matmul(
                        bc_ps,
                        ones_r[D_K:D_K + 1, :],
                        recip[D_K:D_K + 1, :],
                        start=True, stop=True,
                    )
                    bcast = rpool.tile([D_K, QW], F32R, tag="bcast")
                    nc.vector.tensor_copy(bcast, bc_ps)
                    nc.vector.tensor_tensor(
                        ctxT[:, qc * QW:(qc + 1) * QW],
                        ctx_ps[h][0:D_K, :],
                        bcast,
                        mybir.AluOpType.mult,
                    )

                # O-projection for this query chunk (4 token tiles of 128)
                for qt in range(qc * 4, (qc + 1) * 4):
                    o_ps = mm_ps.tile([P, D_MODEL], F32, tag="mm")
                    nc.tensor.matmul(
                        o_ps, ctxT0[:, qt * P:(qt + 1) * P], wo0_sb,
                        start=True, stop=False,
                    )
                    nc.tensor.matmul(
                        o_ps, ctxT1[:, qt * P:(qt + 1) * P], wo1_sb,
                        start=False, stop=True,
                    )
                    o_sb = opool.tile([P, D_MODEL], F32, tag="o")
                    nc.vector.tensor_copy(o_sb, o_ps)
                    nc.sync.dma_start(outp[qt * P:(qt + 1) * P, :], o_sb)

    _hoist_excess_waits(nc)
    return nc


def get_nc(plan):
    if plan not in _BUILD_CACHE:
        _BUILD_CACHE[plan] = _build_nc(plan)
    return _BUILD_CACHE[plan]


def make_in_maps(q, k, v, wq, bq, wk, bk, wv, bv, wo):
    scale = 1.0 / np.sqrt(D_K)
    idstack = np.concatenate([np.eye(D_K, dtype=np.float32)] * 2, axis=0)
    ones_in = np.ones((P, D_K), dtype=np.float32)
    in_maps = []
    for c in range(N_CORES):
        b = c // (N_CORES // B)
        h0 = 2 * (c % (N_CORES // B))
        ds = slice(h0 * D_K, (h0 + 2) * D_K)
        in_maps.append({
            "qT": np.ascontiguousarray(q[b].T),
            "kT": np.ascontiguousarray(k[b].T),
            "vT": np.ascontiguousarray(v[b].T),
            "wqT": np.ascontiguousarray((wq[ds] * scale).T),
            "wkT": np.ascontiguousarray(wk[ds].T),
            "wvT": np.ascontiguousarray(wv[ds].T),
            "bq2": (bq[ds] * scale).reshape(P, 1).astype(np.float32),
            "bk2": bk[ds].reshape(P, 1).astype(np.float32),
            "bv2": bv[ds].reshape(P, 1).astype(np.float32),
            "woT0": np.ascontiguousarray(wo[:, h0 * D_K:(h0 + 1) * D_K].T),
            "woT1": np.ascontiguousarray(wo[:, (h0 + 1) * D_K:(h0 + 2) * D_K].T),
            "idstack": idstack,
            "ones_in": ones_in,
        })
    return in_maps


def classify_mask(mask):
    m = np.asarray(mask)
    if m.all():
        return "full"
    tril = np.tril(np.ones((T, T), dtype=bool))
    if all(np.array_equal(m[b, 0], tril) for b in range(m.shape[0])):
        return "causal"
    return "general"


def _numpy_reference(q, k, v, mask, wq, bq, wk, bk, wv, bv, wo, bo):
    """Fallback for mask patterns the device program doesn't cover."""
    qh = (q @ wq.T + bq).reshape(B, T, N_HEADS, D_K).transpose(0, 2, 1, 3)
    kh = (k @ wk.T + bk).reshape(B, T, N_HEADS, D_K).transpose(0, 2, 1, 3)
    vh = (v @ wv.T + bv).reshape(B, T, N_HEADS, D_K).transpose(0, 2, 1, 3)
    s = np.einsum("bhqd,bhkd->bhqk", qh, kh) / np.sqrt(D_K).astype(np.float32)
    s = np.where(mask, s, -np.inf)
    all_masked = ~mask.any(axis=-1, keepdims=True)
    s = np.where(all_masked, 0.0, s)
    s = s - s.max(axis=-1, keepdims=True)
    e = np.exp(s)
    p = e / e.sum(axis=-1, keepdims=True)
    ctx = np.einsum("bhqk,bhkd->bhqd", p, vh)
    ctx = ctx.transpose(0, 2, 1, 3).reshape(B, T, D_MODEL)
    return (ctx @ wo.T + bo).astype(np.float32)


def kernel(q, k, v, mask, wq, bq, wk, bk, wv, bv, wo, bo, _trace=False):
    q, k, v = (np.asarray(x, dtype=np.float32) for x in (q, k, v))
    mask = np.asarray(mask, dtype=bool)
    wq, bq, wk, bk, wv, bv, wo, bo = (
        np.asarray(x, dtype=np.float32) for x in (wq, bq, wk, bk, wv, bv, wo, bo)
    )

    plan = classify_mask(mask)
    if plan == "general":
        return _numpy_reference(q, k, v, mask, wq, bq, wk, bk, wv, bv, wo, bo)

    nc = get_nc(plan)
    in_maps = make_in_maps(q, k, v, wq, bq, wk, bk, wv, bv, wo)
    res = run_bass_kernel_spmd(
        nc, in_maps, core_ids=list(range(N_CORES)), trace=_trace
    )

    out = np.zeros((B, T, D_MODEL), dtype=np.float32)
    for c in range(N_CORES):
        out[c // (N_CORES // B)] += res.results[c]["outp"]
    out += bo[None, None, :]
    if _trace:
        kernel.last_exec_time_ns = res.exec_time_ns
        kernel.last_res = res
    return out


# revision 6
# speedup vs baseline: 1.1485x; 1.0454x over previous
"""MultiHeadAttention (B=2, T=2048, D=512, H=8, causal) on 8 trn2 NeuronCores.

Sharding: batch*heads across cores. Core c handles batch c//4 and heads
{2*(c%4), 2*(c%4)+1}. Each core projects Q/K/V for its two heads (weight
slices replicated), runs softmax attention with scores materialized
transposed ([keys, queries] so the softmax reduction lands on the PSUM/matmul
path instead of cross-partition ops), applies its slice of the output
projection, and writes a [T, D] partial. Host sums the 8 partials (+ output
bias) into the full [B, T, D] result.

Numerics: all large matmuls run in float32r (TF32-like, ~1.2e-4 relative),
exact-fp32 PE transposes for V, softmax without max-subtraction (scores are
O(1) by construction: Q/K projections of unit-variance data through
U(-1/sqrt(D)) weights; exp stays far from fp32 range).
"""

import numpy as np

import concourse.bass as bass
import concourse.mybir as mybir
import concourse.tile as tile
from concourse.bass_utils import run_bass_kernel_spmd

D_MODEL = 512
N_HEADS = 8
D_K = 64
B = 2
T = 2048
N_CORES = 8
P = 128
QC = 4           # query chunks of 512
QW = T // QC     # 512 queries per chunk
KT = T // P      # 16 key tiles of 128
F32 = mybir.dt.float32
F32R = mybir.dt.float32r
BF16 = mybir.dt.bfloat16

# dtype configuration (module-level; set before first kernel() call)
ATTN_DT = "f32r"   # "f32r" | "bf16"  -- scores/AV operand dtype
IN_DT = "f32"      # "f32" | "bf16"   -- host->device x + projection dtype

_BUILD_CACHE = {}


def _hoist_excess_waits(nc, max_waits=1):
    """walrus codegen supports at most one sync-wait slot per hardware
    instruction, but Tile's sem-assignment can attach several (e.g. inputs
    arriving via two HW-DGE queues). Move the excess onto same-engine no-ops
    placed just before the instruction."""
    n_fixed = 0
    for fn in nc.m.functions:
        for bb in fn.blocks:
            insts = bb.instructions
            new_list = []
            for ins in insts:
                si = ins.sync_info
                ow = list(si.on_wait or []) if si else []
                if len(ow) > max_waits and ins.is_executable():
                    for j, w in enumerate(ow[max_waits:]):
                        nop = mybir.InstNoOp(
                            name=f"waitnop{j}_{ins.name}", ins=[], outs=[]
                        )
                        nop.engine = ins.engine
                        nop.sync_info = mybir.SyncInfo(on_wait=[w], on_update=[])
                        new_list.append(nop)
                    si.on_wait = ow[:max_waits]
                    ins.sync_info = si
                    n_fixed += 1
                new_list.append(ins)
            insts[:] = new_list
    return n_fixed


def _build_nc(plan, attn_dt, in_dt):
    """plan: 'causal' or 'full' -> one SPMD program for all 8 cores."""
    nc = bass.Bass(target_bir_lowering=False)
    ADT = BF16 if attn_dt == "bf16" else F32R
    XDT = BF16 if in_dt == "bf16" else F32
    XSB = BF16 if in_dt == "bf16" else F32R

    qT = nc.dram_tensor("qT", [D_MODEL, T], XDT, kind="ExternalInput")
    kT = nc.dram_tensor("kT", [D_MODEL, T], XDT, kind="ExternalInput")
    vT = nc.dram_tensor("vT", [D_MODEL, T], XDT, kind="ExternalInput")
    wqT = nc.dram_tensor("wqT", [D_MODEL, P], XDT, kind="ExternalInput")
    wkT = nc.dram_tensor("wkT", [D_MODEL, P], XDT, kind="ExternalInput")
    wvT = nc.dram_tensor("wvT", [D_MODEL, P], XDT, kind="ExternalInput")
    bq2 = nc.dram_tensor("bq2", [P, 1], F32, kind="ExternalInput")
    bk2 = nc.dram_tensor("bk2", [P, 1], F32, kind="ExternalInput")
    bv2 = nc.dram_tensor("bv2", [P, 1], F32, kind="ExternalInput")
    woT0 = nc.dram_tensor("woT0", [D_K, D_MODEL], F32, kind="ExternalInput")
    woT1 = nc.dram_tensor("woT1", [D_K, D_MODEL], F32, kind="ExternalInput")
    idstack = nc.dram_tensor("idstack", [P, D_K], F32, kind="ExternalInput")
    ones_in = nc.dram_tensor("ones_in", [P, D_K], F32, kind="ExternalInput")
    outp = nc.dram_tensor("outp", [T, D_MODEL], F32, kind="ExternalOutput")

    def kts_of(qc):
        return list(range(KT if plan == "full" else 4 * (qc + 1)))

    def is_partial(qc, kt):
        return plan == "causal" and 4 * qc <= kt <= 4 * qc + 3

    with tile.TileContext(nc) as tc:
        with (
            tc.tile_pool(name="consts", bufs=1) as consts,
            tc.tile_pool(name="xin", bufs=4) as xin,
            tc.tile_pool(name="projT", bufs=1) as projT,
            tc.tile_pool(name="epool", bufs=6) as epool,
            tc.tile_pool(name="rpool", bufs=2) as rpool,
            tc.tile_pool(name="opool", bufs=3) as opool,
            tc.tile_pool(name="mm_ps", bufs=4, space="PSUM") as mm_ps,
            tc.tile_pool(name="ctx_ps", bufs=2, space="PSUM") as ctx_ps_pool,
        ):
            # ---- constants ----
            wq_sb = consts.tile([P, 4, P], XSB, tag="wq")
            wk_sb = consts.tile([P, 4, P], XSB, tag="wk")
            wv_sb = consts.tile([P, 4, P], XSB, tag="wv")
            nc.sync.dma_start(wq_sb, wqT.ap().bitcast(XSB).rearrange("(c p) m -> p c m", p=P))
            nc.sync.dma_start(wk_sb, wkT.ap().bitcast(XSB).rearrange("(c p) m -> p c m", p=P))
            nc.sync.dma_start(wv_sb, wvT.ap().bitcast(XSB).rearrange("(c p) m -> p c m", p=P))
            bq_sb = consts.tile([P, 1], F32, tag="bq")
            bk_sb = consts.tile([P, 1], F32, tag="bk")
            bv_sb = consts.tile([P, 1], F32, tag="bv")
            nc.sync.dma_start(bq_sb, bq2[:, :])
            nc.sync.dma_start(bk_sb, bk2[:, :])
            nc.sync.dma_start(bv_sb, bv2[:, :])
            wo0_sb = consts.tile([D_K, D_MODEL], F32R, tag="wo0")
            wo1_sb = consts.tile([D_K, D_MODEL], F32R, tag="wo1")
            nc.sync.dma_start(wo0_sb, woT0.ap().bitcast(F32R))
            nc.sync.dma_start(wo1_sb, woT1.ap().bitcast(F32R))
            ident = consts.tile([P, D_K], F32, tag="ident")
            nc.sync.dma_start(ident, idstack[:, :])
            ones_r = consts.tile([P, D_K], F32R, tag="ones")
            nc.sync.dma_start(ones_r, ones_in.ap().bitcast(F32R))

            # ---- projections ----
            qhT = projT.tile([P, T], ADT, tag="qhT")   # [2*dk, T]
            khT = projT.tile([P, T], ADT, tag="khT")
            vhT = projT.tile([P, T], F32, tag="vhT")    # fp32: transposed exactly

            vaug = projT.tile([P, KT, 2, D_K + 1], ADT, tag="vaug")

            def project_chunk(xT_dram, w_sb, b_sb, dst, qc, xname):
                x_sb = xin.tile([P, 4, QW], XSB, tag="xin", name=f"x_{xname}_{qc}")
                nc.sync.dma_start(
                    x_sb,
                    xT_dram.ap()
                    .bitcast(XSB)
                    .rearrange("(c p) t -> p c t", p=P)[:, :, qc * QW:(qc + 1) * QW],
                )
                ps = mm_ps.tile([P, QW], F32, tag="mm", name=f"ps_{xname}_{qc}")
                for kc in range(4):
                    nc.tensor.matmul(
                        ps, w_sb[:, kc, :], x_sb[:, kc, :],
                        start=(kc == 0), stop=(kc == 3),
                    )
                nc.vector.tensor_scalar(
                    out=dst[:, qc * QW:(qc + 1) * QW],
                    in0=ps, scalar1=b_sb, scalar2=None,
                    op0=mybir.AluOpType.add,
                )

            def vaug_chunk(qc):
                # transpose v-chunk qc's 4 key tiles into vaug (exact fp32)
                for kt in range(4 * qc, 4 * qc + 4):
                    for h in range(2):
                        tr = mm_ps.tile([P, D_K], F32, tag="mm", name=f"tr_{kt}_{h}")
                        nc.tensor.transpose(
                            tr,
                            vhT[h * D_K:(h + 1) * D_K, kt * P:(kt + 1) * P],
                            ident[h * D_K:(h + 1) * D_K, :],
                        )
                        nc.vector.tensor_copy(vaug[:, kt, h, 0:D_K], tr)
                        nc.vector.tensor_copy(vaug[:, kt, h, D_K:D_K + 1], ones_r[:, 0:1])

            # chunk order: unblock attention qc=0 (needs k0,v0,q0) asap
            project_chunk(kT, wk_sb, bk_sb, khT, 0, "k")
            project_chunk(qT, wq_sb, bq_sb, qhT, 0, "q")
            project_chunk(vT, wv_sb, bv_sb, vhT, 0, "v")
            vaug_chunk(0)
            for qc in range(1, QC):
                project_chunk(kT, wk_sb, bk_sb, khT, qc, "k")
                project_chunk(vT, wv_sb, bv_sb, vhT, qc, "v")
                project_chunk(qT, wq_sb, bq_sb, qhT, qc, "q")
                vaug_chunk(qc)

            # ---- attention + output projection ----
            ctxT0 = projT.tile([D_K, T], F32R, tag="ctxT0")
            ctxT1 = projT.tile([D_K, T], F32R, tag="ctxT1")

            for qc in range(QC):
                kts = kts_of(qc)
                ctx_ps = [
                    ctx_ps_pool.tile([D_K + 1, QW], F32, tag="ctx", name=f"ctx_{qc}_{h}")
                    for h in range(2)
                ]
                for kt in kts:
                    for h in range(2):
                        hs = slice(h * D_K, (h + 1) * D_K)
                        s_ps = mm_ps.tile([P, QW], F32, tag="mm")
                        nc.tensor.matmul(
                            s_ps,
                            khT[hs, kt * P:(kt + 1) * P],
                            qhT[hs, qc * QW:(qc + 1) * QW],
                            start=True, stop=True,
                        )
                        e_sb = epool.tile([P, QW], ADT, tag="e")
                        nc.scalar.activation(
                            e_sb, s_ps, mybir.ActivationFunctionType.Exp
                        )
                        if is_partial(qc, kt):
                            nc.gpsimd.affine_select(
                                out=e_sb, in_=e_sb,
                                compare_op=mybir.AluOpType.is_ge,
                                fill=0.0,
                                base=qc * QW - kt * P,
                                pattern=[[1, QW]],
                                channel_multiplier=-1,
                            )
                        nc.tensor.matmul(
                            ctx_ps[h],
                            vaug[:, kt, h, :],
                            e_sb,
                            start=(kt == kts[0]), stop=(kt == kts[-1]),
                        )
                for h, ctxT in ((0, ctxT0), (1, ctxT1)):
                    recip = rpool.tile([D_K + 1, QW], F32R, tag="recip")
                    lnd = rpool.tile([D_K + 1, QW], F32, tag="lnd")
                    nc.scalar.activation(
                        lnd[D_K:D_K + 1, :], ctx_ps[h][D_K:D_K + 1, :],
                        mybir.ActivationFunctionType.Ln,
                    )
                    nc.scalar.activation(
                        recip[D_K:D_K + 1, :], lnd[D_K:D_K + 1, :],
                        mybir.ActivationFunctionType.Exp, scale=-1.0,
                    )
                    bc_ps = mm_ps.tile([D_K, QW], F32, tag="mm")
                    nc.tensor.matmul(
                        bc_ps,
                        ones_r[D_K:D_K + 1, :],
                        recip[D_K:D_K + 1, :],
                        start=True, stop=True,
                    )
                    bcast = rpool.tile([D_K, QW], F32R, tag="bcast")
                    nc.vector.tensor_copy(bcast, bc_ps)
                    nc.vector.tensor_tensor(
                        ctxT[:, qc * QW:(qc + 1) * QW],
                        ctx_ps[h][0:D_K, :],
                        bcast,
                        mybir.AluOpType.mult,
                    )

                # O-projection for this query chunk (4 token tiles of 128)
                for qt in range(qc * 4, (qc + 1) * 4):
                    o_ps = mm_ps.tile([P, D_MODEL], F32, tag="mm")
                    nc.tensor.matmul(
                        o_ps, ctxT0[:, qt * P:(qt + 1) * P], wo0_sb,
                        start=True, stop=False,
                    )
                    nc.tensor.matmul(
                        o_ps, ctxT1[:, qt * P:(qt + 1) * P], wo1_sb,
                        start=False, stop=True,
                    )
                    o_sb = opool.tile([P, D_MODEL], F32, tag="o")
                    nc.vector.tensor_copy(o_sb, o_ps)
                    nc.sync.dma_start(outp[qt * P:(qt + 1) * P, :], o_sb)

    _hoist_excess_waits(nc)
    return nc


def get_nc(plan):
    key = (plan, ATTN_DT, IN_DT)
    if key not in _BUILD_CACHE:
        _BUILD_CACHE[key] = _build_nc(plan, ATTN_DT, IN_DT)
    return _BUILD_CACHE[key]


def make_in_maps(q, k, v, wq, bq, wk, bk, wv, bv, wo):
    scale = 1.0 / np.sqrt(D_K)
    if IN_DT == "bf16":
        import ml_dtypes
        xdt = ml_dtypes.bfloat16
    else:
        xdt = np.float32
    idstack = np.concatenate([np.eye(D_K, dtype=np.float32)] * 2, axis=0)
    ones_in = np.ones((P, D_K), dtype=np.float32)
    in_maps = []
    for c in range(N_CORES):
        b = c // (N_CORES // B)
        h0 = 2 * (c % (N_CORES // B))
        ds = slice(h0 * D_K, (h0 + 2) * D_K)
        in_maps.append({
            "qT": np.ascontiguousarray(q[b].T).astype(xdt),
            "kT": np.ascontiguousarray(k[b].T).astype(xdt),
            "vT": np.ascontiguousarray(v[b].T).astype(xdt),
            "wqT": np.ascontiguousarray((wq[ds] * scale).T).astype(xdt),
            "wkT": np.ascontiguousarray(wk[ds].T).astype(xdt),
            "wvT": np.ascontiguousarray(wv[ds].T).astype(xdt),
            "bq2": (bq[ds] * scale).reshape(P, 1).astype(np.float32),
            "bk2": bk[ds].reshape(P, 1).astype(np.float32),
            "bv2": bv[ds].reshape(P, 1).astype(np.float32),
            "woT0": np.ascontiguousarray(wo[:, h0 * D_K:(h0 + 1) * D_K].T),
            "woT1": np.ascontiguousarray(wo[:, (h0 + 1) * D_K:(h0 + 2) * D_K].T),
            "idstack": idstack,
            "ones_in": ones_in,
        })
    return in_maps


def classify_mask(mask):
    m = np.asarray(mask)
    if m.all():
        return "full"
    tril = np.tril(np.ones((T, T), dtype=bool))
    if all(np.array_equal(m[b, 0], tril) for b in range(m.shape[0])):
        return "causal"
    return "general"


def _numpy_reference(q, k, v, mask, wq, bq, wk, bk, wv, bv, wo, bo):
    """Fallback for mask patterns the device program doesn't cover."""
    qh = (q @ wq.T + bq).reshape(B, T, N_HEADS, D_K).transpose(0, 2, 1, 3)
    kh = (k @ wk.T + bk).reshape(B, T, N_HEADS, D_K).transpose(0, 2, 1, 3)
    vh = (v @ wv.T + bv).reshape(B, T, N_HEADS, D_K).transpose(0, 2, 1, 3)
    s = np.einsum("bhqd,bhkd->bhqk", qh, kh) / np.sqrt(D_K).astype(np.float32)
    s = np.where(mask, s, -np.inf)
    all_masked = ~mask.any(axis=-1, keepdims=True)
    s = np.where(all_masked, 0.0, s)
    s = s - s.max(axis=-1, keepdims=True)
    e = np.exp(s)
    p = e / e.sum(axis=-1, keepdims=True)
    ctx = np.einsum("bhqk,bhkd->bhqd", p, vh)
    ctx = ctx.transpose(0, 2, 1, 3).reshape(B, T, D_MODEL)
    return (ctx @ wo.T + bo).astype(np.float32)


def kernel(q, k, v, mask, wq, bq, wk, bk, wv, bv, wo, bo, _trace=False):
    q, k, v = (np.asarray(x, dtype=np.float32) for x in (q, k, v))
    mask = np.asarray(mask, dtype=bool)
    wq, bq, wk, bk, wv, bv, wo, bo = (
        np.asarray(x, dtype=np.float32) for x in (wq, bq, wk, bk, wv, bv, wo, bo)
    )

    plan = classify_mask(mask)
    if plan == "general":
        return _numpy_reference(q, k, v, mask, wq, bq, wk, bk, wv, bv, wo, bo)

    nc = get_nc(plan)
    in_maps = make_in_maps(q, k, v, wq, bq, wk, bk, wv, bv, wo)
    res = run_bass_kernel_spmd(
        nc, in_maps, core_ids=list(range(N_CORES)), trace=_trace
    )

    out = np.zeros((B, T, D_MODEL), dtype=np.float32)
    for c in range(N_CORES):
        out[c // (N_CORES // B)] += res.results[c]["outp"]
    out += bo[None, None, :]
    if _trace:
        kernel.last_exec_time_ns = res.exec_time_ns
        kernel.last_res = res
    return out


# revision 7
# speedup vs baseline: 1.2061x; 1.0501x over previous
"""MultiHeadAttention (B=2, T=2048, D=512, H=8, causal) on 8 trn2 NeuronCores.

Sharding: batch*heads across cores. Core c handles batch c//4 and heads
{2*(c%4), 2*(c%4)+1}. Each core projects Q/K/V for its two heads (weight
slices replicated), runs softmax attention with scores materialized
transposed ([keys, queries] so the softmax reduction lands on the PSUM/matmul
path instead of cross-partition ops), applies its slice of the output
projection, and writes a [T, D] partial. Host sums the 8 partials (+ output
bias) into the full [B, T, D] result.

Numerics: all large matmuls run in float32r (TF32-like, ~1.2e-4 relative),
exact-fp32 PE transposes for V, softmax without max-subtraction (scores are
O(1) by construction: Q/K projections of unit-variance data through
U(-1/sqrt(D)) weights; exp stays far from fp32 range).
"""

import numpy as np

import concourse.bass as bass
import concourse.mybir as mybir
import concourse.tile as tile
from concourse.bass_utils import run_bass_kernel_spmd

D_MODEL = 512
N_HEADS = 8
D_K = 64
B = 2
T = 2048
N_CORES = 8
P = 128
QC = 4           # query chunks of 512
QW = T // QC     # 512 queries per chunk
KT = T // P      # 16 key tiles of 128
F32 = mybir.dt.float32
F32R = mybir.dt.float32r
BF16 = mybir.dt.bfloat16

# dtype configuration (module-level; set before first kernel() call)
ATTN_DT = "f32r"   # "f32r" | "bf16"  -- scores/AV operand dtype
IN_DT = "f32"      # "f32" | "bf16"   -- host->device x + projection dtype

_BUILD_CACHE = {}


def _hoist_excess_waits(nc, max_waits=1):
    """walrus codegen supports at most one sync-wait slot per hardware
    instruction, but Tile's sem-assignment can attach several (e.g. inputs
    arriving via two HW-DGE queues). Move the excess onto same-engine no-ops
    placed just before the instruction."""
    n_fixed = 0
    for fn in nc.m.functions:
        for bb in fn.blocks:
            insts = bb.instructions
            new_list = []
            for ins in insts:
                si = ins.sync_info
                ow = list(si.on_wait or []) if si else []
                if len(ow) > max_waits and ins.is_executable():
                    for j, w in enumerate(ow[max_waits:]):
                        nop = mybir.InstNoOp(
                            name=f"waitnop{j}_{ins.name}", ins=[], outs=[]
                        )
                        nop.engine = ins.engine
                        nop.sync_info = mybir.SyncInfo(on_wait=[w], on_update=[])
                        new_list.append(nop)
                    si.on_wait = ow[:max_waits]
                    ins.sync_info = si
                    n_fixed += 1
                new_list.append(ins)
            insts[:] = new_list
    return n_fixed


def _build_nc(plan, attn_dt, in_dt):
    """plan: 'causal' or 'full' -> one SPMD program for all 8 cores."""
    nc = bass.Bass(target_bir_lowering=False)
    ADT = BF16 if attn_dt == "bf16" else F32R
    XDT = BF16 if in_dt == "bf16" else F32
    XSB = BF16 if in_dt == "bf16" else F32R

    qT = nc.dram_tensor("qT", [P, 4, T], XDT, kind="ExternalInput")
    kT = nc.dram_tensor("kT", [P, 4, T], XDT, kind="ExternalInput")
    vT = nc.dram_tensor("vT", [P, 4, T], XDT, kind="ExternalInput")
    wqT = nc.dram_tensor("wqT", [D_MODEL, P], XDT, kind="ExternalInput")
    wkT = nc.dram_tensor("wkT", [D_MODEL, P], XDT, kind="ExternalInput")
    wvT = nc.dram_tensor("wvT", [D_MODEL, P], XDT, kind="ExternalInput")
    bq2 = nc.dram_tensor("bq2", [P, 1], F32, kind="ExternalInput")
    bk2 = nc.dram_tensor("bk2", [P, 1], F32, kind="ExternalInput")
    bv2 = nc.dram_tensor("bv2", [P, 1], F32, kind="ExternalInput")
    woT0 = nc.dram_tensor("woT0", [D_K, D_MODEL], F32, kind="ExternalInput")
    woT1 = nc.dram_tensor("woT1", [D_K, D_MODEL], F32, kind="ExternalInput")
    idstack = nc.dram_tensor("idstack", [P, D_K], F32, kind="ExternalInput")
    ones_in = nc.dram_tensor("ones_in", [P, D_K], F32, kind="ExternalInput")
    outp = nc.dram_tensor("outp", [T, D_MODEL], F32, kind="ExternalOutput")

    def kts_of(qc):
        return list(range(KT if plan == "full" else 4 * (qc + 1)))

    def is_partial(qc, kt):
        return plan == "causal" and 4 * qc <= kt <= 4 * qc + 3

    with tile.TileContext(nc) as tc:
        with (
            tc.tile_pool(name="consts", bufs=1) as consts,
            tc.tile_pool(name="xin", bufs=1) as xin,
            tc.tile_pool(name="projT", bufs=1) as projT,
            tc.tile_pool(name="epool", bufs=4) as epool,
            tc.tile_pool(name="rpool", bufs=2) as rpool,
            tc.tile_pool(name="opool", bufs=3) as opool,
            tc.tile_pool(name="mm_ps", bufs=2, space="PSUM") as mm_ps,
            tc.tile_pool(name="s2_ps", bufs=2, space="PSUM") as s2_ps,
            tc.tile_pool(name="ctx_ps", bufs=2, space="PSUM") as ctx_ps_pool,
        ):
            # ---- constants ----
            wq_sb = consts.tile([P, 4, P], XSB, tag="wq")
            wk_sb = consts.tile([P, 4, P], XSB, tag="wk")
            wv_sb = consts.tile([P, 4, P], XSB, tag="wv")
            nc.sync.dma_start(wq_sb, wqT.ap().bitcast(XSB).rearrange("(c p) m -> p c m", p=P))
            nc.sync.dma_start(wk_sb, wkT.ap().bitcast(XSB).rearrange("(c p) m -> p c m", p=P))
            nc.sync.dma_start(wv_sb, wvT.ap().bitcast(XSB).rearrange("(c p) m -> p c m", p=P))
            bq_sb = consts.tile([P, 1], F32, tag="bq")
            bk_sb = consts.tile([P, 1], F32, tag="bk")
            bv_sb = consts.tile([P, 1], F32, tag="bv")
            nc.sync.dma_start(bq_sb, bq2[:, :])
            nc.sync.dma_start(bk_sb, bk2[:, :])
            nc.sync.dma_start(bv_sb, bv2[:, :])
            wo0_sb = consts.tile([D_K, D_MODEL], F32R, tag="wo0")
            wo1_sb = consts.tile([D_K, D_MODEL], F32R, tag="wo1")
            nc.sync.dma_start(wo0_sb, woT0.ap().bitcast(F32R))
            nc.sync.dma_start(wo1_sb, woT1.ap().bitcast(F32R))
            ident = consts.tile([P, D_K], F32, tag="ident")
            nc.sync.dma_start(ident, idstack[:, :])
            ones_r = consts.tile([P, D_K], F32R, tag="ones")
            nc.sync.dma_start(ones_r, ones_in.ap().bitcast(F32R))

            # ---- projections ----
            qhT = projT.tile([P, T], ADT, tag="qhT")   # [2*dk, T]
            khT = projT.tile([P, T], ADT, tag="khT")
            vhT = projT.tile([P, T], F32, tag="vhT")    # fp32: transposed exactly

            vaug = projT.tile([P, KT, 2, D_K + 1], ADT, tag="vaug")

            x_tiles = {}

            def load_x(xT_dram, xname):
                x_sb = xin.tile([P, 4, T], XSB, tag=f"x_{xname}", name=f"x_{xname}")
                for kc in range(4):
                    nc.sync.dma_start(
                        x_sb[:, kc, :], xT_dram.ap().bitcast(XSB)[:, kc, :]
                    )
                x_tiles[xname] = x_sb

            def project_chunk(xname, w_sb, b_sb, dst, qc):
                x_sb = x_tiles[xname]
                ps = mm_ps.tile([P, QW], F32, tag="mm", name=f"ps_{xname}_{qc}")
                for kc in range(4):
                    nc.tensor.matmul(
                        ps, w_sb[:, kc, :], x_sb[:, kc, qc * QW:(qc + 1) * QW],
                        start=(kc == 0), stop=(kc == 3),
                    )
                nc.vector.tensor_scalar(
                    out=dst[:, qc * QW:(qc + 1) * QW],
                    in0=ps, scalar1=b_sb, scalar2=None,
                    op0=mybir.AluOpType.add,
                )

            def vaug_chunk(qc):
                # transpose v-chunk qc's 4 key tiles into vaug (exact fp32)
                for kt in range(4 * qc, 4 * qc + 4):
                    for h in range(2):
                        tr = mm_ps.tile([P, D_K], F32, tag="mm", name=f"tr_{kt}_{h}")
                        nc.tensor.transpose(
                            tr,
                            vhT[h * D_K:(h + 1) * D_K, kt * P:(kt + 1) * P],
                            ident[h * D_K:(h + 1) * D_K, :],
                        )
                        nc.vector.tensor_copy(vaug[:, kt, h, 0:D_K], tr)
                        nc.vector.tensor_copy(vaug[:, kt, h, D_K:D_K + 1], ones_r[:, 0:1])

            # load whole tensors (128 x 8KB descriptors per DMA), then project
            load_x(kT, "k")
            load_x(qT, "q")
            load_x(vT, "v")
            project_chunk("k", wk_sb, bk_sb, khT, 0)
            project_chunk("q", wq_sb, bq_sb, qhT, 0)
            project_chunk("v", wv_sb, bv_sb, vhT, 0)
            vaug_chunk(0)
            for qc in range(1, QC):
                project_chunk("k", wk_sb, bk_sb, khT, qc)
                project_chunk("v", wv_sb, bv_sb, vhT, qc)
                project_chunk("q", wq_sb, bq_sb, qhT, qc)
                vaug_chunk(qc)

            # ---- attention + output projection ----
            ctxT0 = projT.tile([D_K, T], F32R, tag="ctxT0")
            ctxT1 = projT.tile([D_K, T], F32R, tag="ctxT1")

            for qc in range(QC):
                kts = kts_of(qc)
                ctx_ps = [
                    ctx_ps_pool.tile([D_K + 1, QW], F32, tag="ctx", name=f"ctx_{qc}_{h}")
                    for h in range(2)
                ]
                for kp in range(0, len(kts), 2):
                    pair = kts[kp:kp + 2]
                    for h in range(2):
                        hs = slice(h * D_K, (h + 1) * D_K)
                        s_ps = s2_ps.tile([P, 2, QW], F32, tag="s2")
                        for j, kt in enumerate(pair):
                            nc.tensor.matmul(
                                s_ps[:, j, :],
                                khT[hs, kt * P:(kt + 1) * P],
                                qhT[hs, qc * QW:(qc + 1) * QW],
                                start=True, stop=True,
                            )
                        e_sb = epool.tile([P, 2, QW], ADT, tag="e")
                        nc.scalar.activation(
                            e_sb[:, :len(pair), :], s_ps[:, :len(pair), :],
                            mybir.ActivationFunctionType.Exp,
                        )
                        for j, kt in enumerate(pair):
                            if is_partial(qc, kt):
                                nc.gpsimd.affine_select(
                                    out=e_sb[:, j, :], in_=e_sb[:, j, :],
                                    compare_op=mybir.AluOpType.is_ge,
                                    fill=0.0,
                                    base=qc * QW - kt * P,
                                    pattern=[[1, QW]],
                                    channel_multiplier=-1,
                                )
                            nc.tensor.matmul(
                                ctx_ps[h],
                                vaug[:, kt, h, :],
                                e_sb[:, j, :],
                                start=(kt == kts[0]), stop=(kt == kts[-1]),
                            )
                for h, ctxT in ((0, ctxT0), (1, ctxT1)):
                    recip = rpool.tile([D_K + 1, QW], F32R, tag="recip")
                    lnd = rpool.tile([D_K + 1, QW], F32, tag="lnd")
                    nc.scalar.activation(
                        lnd[D_K:D_K + 1, :], ctx_ps[h][D_K:D_K + 1, :],
                        mybir.ActivationFunctionType.Ln,
                    )
                    nc.scalar.activation(
                        recip[D_K:D_K + 1, :], lnd[D_K:D_K + 1, :],
                        mybir.ActivationFunctionType.Exp, scale=-1.0,
                    )
                    bc_ps = mm_ps.tile([D_K, QW], F32, tag="mm")
                    nc.tensor.matmul(
                        bc_ps,
                        ones_r[D_K:D_K + 1, :],
                        recip[D_K:D_K + 1, :],
                        start=True, stop=True,
                    )
                    bcast = rpool.tile([D_K, QW], F32R, tag="bcast")
                    nc.vector.tensor_copy(bcast, bc_ps)
                    nc.vector.tensor_tensor(
                        ctxT[:, qc * QW:(qc + 1) * QW],
                        ctx_ps[h][0:D_K, :],
                        bcast,
                        mybir.AluOpType.mult,
                    )

                # O-projection for this query chunk (4 token tiles of 128)
                for qt in range(qc * 4, (qc + 1) * 4):
                    o_ps = mm_ps.tile([P, D_MODEL], F32, tag="mm")
                    nc.tensor.matmul(
                        o_ps, ctxT0[:, qt * P:(qt + 1) * P], wo0_sb,
                        start=True, stop=False,
                    )
                    nc.tensor.matmul(
                        o_ps, ctxT1[:, qt * P:(qt + 1) * P], wo1_sb,
                        start=False, stop=True,
                    )
                    o_sb = opool.tile([P, D_MODEL], F32, tag="o")
                    nc.vector.tensor_copy(o_sb, o_ps)
                    nc.sync.dma_start(outp[qt * P:(qt + 1) * P, :], o_sb)

    _hoist_excess_waits(nc)
    return nc


def get_nc(plan):
    key = (plan, ATTN_DT, IN_DT)
    if key not in _BUILD_CACHE:
        _BUILD_CACHE[key] = _build_nc(plan, ATTN_DT, IN_DT)
    return _BUILD_CACHE[key]


def make_in_maps(q, k, v, wq, bq, wk, bk, wv, bv, wo):
    scale = 1.0 / np.sqrt(D_K)
    if IN_DT == "bf16":
        import ml_dtypes
        xdt = ml_dtypes.bfloat16
    else:
        xdt = np.float32
    idstack = np.concatenate([np.eye(D_K, dtype=np.float32)] * 2, axis=0)
    ones_in = np.ones((P, D_K), dtype=np.float32)
    def interleave(x):
        # [T, D] -> x.T grouped as [128, 4, T]: row kc*128+p of x.T at [p, kc]
        return np.ascontiguousarray(
            x.T.reshape(4, P, T).transpose(1, 0, 2)
        ).astype(xdt)

    in_maps = []
    for c in range(N_CORES):
        b = c // (N_CORES // B)
        h0 = 2 * (c % (N_CORES // B))
        ds = slice(h0 * D_K, (h0 + 2) * D_K)
        in_maps.append({
            "qT": interleave(q[b]),
            "kT": interleave(k[b]),
            "vT": interleave(v[b]),
            "wqT": np.ascontiguousarray((wq[ds] * scale).T).astype(xdt),
            "wkT": np.ascontiguousarray(wk[ds].T).astype(xdt),
            "wvT": np.ascontiguousarray(wv[ds].T).astype(xdt),
            "bq2": (bq[ds] * scale).reshape(P, 1).astype(np.float32),
            "bk2": bk[ds].reshape(P, 1).astype(np.float32),
            "bv2": bv[ds].reshape(P, 1).astype(np.float32),
            "woT0": np.ascontiguousarray(wo[:, h0 * D_K:(h0 + 1) * D_K].T),
            "woT1": np.ascontiguousarray(wo[:, (h0 + 1) * D_K:(h0 + 2) * D_K].T),
            "idstack": idstack,
            "ones_in": ones_in,
        })
    return in_maps


def classify_mask(mask):
    m = np.asarray(mask)
    if m.all():
        return "full"
    tril = np.tril(np.ones((T, T), dtype=bool))
    if all(np.array_equal(m[b, 0], tril) for b in range(m.shape[0])):
        return "causal"
    return "general"


def _numpy_reference(q, k, v, mask, wq, bq, wk, bk, wv, bv, wo, bo):
    """Fallback for mask patterns the device program doesn't cover."""
    qh = (q @ wq.T + bq).reshape(B, T, N_HEADS, D_K).transpose(0, 2, 1, 3)
    kh = (k @ wk.T + bk).reshape(B, T, N_HEADS, D_K).transpose(0, 2, 1, 3)
    vh = (v @ wv.T + bv).reshape(B, T, N_HEADS, D_K).transpose(0, 2, 1, 3)
    s = np.einsum("bhqd,bhkd->bhqk", qh, kh) / np.sqrt(D_K).astype(np.float32)
    s = np.where(mask, s, -np.inf)
    all_masked = ~mask.any(axis=-1, keepdims=True)
    s = np.where(all_masked, 0.0, s)
    s = s - s.max(axis=-1, keepdims=True)
    e = np.exp(s)
    p = e / e.sum(axis=-1, keepdims=True)
    ctx = np.einsum("bhqk,bhkd->bhqd", p, vh)
    ctx = ctx.transpose(0, 2, 1, 3).reshape(B, T, D_MODEL)
    return (ctx @ wo.T + bo).astype(np.float32)


def kernel(q, k, v, mask, wq, bq, wk, bk, wv, bv, wo, bo, _trace=False):
    q, k, v = (np.asarray(x, dtype=np.float32) for x in (q, k, v))
    mask = np.asarray(mask, dtype=bool)
    wq, bq, wk, bk, wv, bv, wo, bo = (
        np.asarray(x, dtype=np.float32) for x in (wq, bq, wk, bk, wv, bv, wo, bo)
    )

    plan = classify_mask(mask)
    if plan == "general":
        return _numpy_reference(q, k, v, mask, wq, bq, wk, bk, wv, bv, wo, bo)

    nc = get_nc(plan)
    in_maps = make_in_maps(q, k, v, wq, bq, wk, bk, wv, bv, wo)
    res = run_bass_kernel_spmd(
        nc, in_maps, core_ids=list(range(N_CORES)), trace=_trace
    )

    out = np.zeros((B, T, D_MODEL), dtype=np.float32)
    for c in range(N_CORES):
        out[c // (N_CORES // B)] += res.results[c]["outp"]
    out += bo[None, None, :]
    if _trace:
        kernel.last_exec_time_ns = res.exec_time_ns
        kernel.last_res = res
    return out


# revision 9
# speedup vs baseline: 1.4112x; 1.1701x over previous
"""MultiHeadAttention (B=2, T=2048, D=512, H=8, causal) on 8 trn2 NeuronCores.

Sharding: batch*heads across cores. Core c handles batch c//4 and heads
{2*(c%4), 2*(c%4)+1}. Each core projects Q/K/V for its two heads (weight
slices replicated), runs softmax attention with scores materialized
transposed ([keys, queries] so the softmax reduction lands on the PSUM/matmul
path instead of cross-partition ops), applies its slice of the output
projection, and writes a [T, D] partial. Host sums the 8 partials (+ output
bias) into the full [B, T, D] result.

Numerics: all large matmuls run in float32r (TF32-like, ~1.2e-4 relative),
exact-fp32 PE transposes for V, softmax without max-subtraction (scores are
O(1) by construction: Q/K projections of unit-variance data through
U(-1/sqrt(D)) weights; exp stays far from fp32 range).
"""

import numpy as np

import concourse.bass as bass
import concourse.mybir as mybir
import concourse.tile as tile
from concourse.bass_utils import run_bass_kernel_spmd

D_MODEL = 512
N_HEADS = 8
D_K = 64
B = 2
T = 2048
N_CORES = 8
P = 128
QC = 4           # query chunks of 512
QW = T // QC     # 512 queries per chunk
KT = T // P      # 16 key tiles of 128
F32 = mybir.dt.float32
F32R = mybir.dt.float32r
BF16 = mybir.dt.bfloat16

# dtype configuration (module-level; set before first kernel() call)
ATTN_DT = "f32r"   # "f32r" | "bf16"  -- scores/AV operand dtype
IN_DT = "f32"      # "f32" | "bf16"   -- host->device x + projection dtype

_BUILD_CACHE = {}


def _hoist_excess_waits(nc, max_waits=1):
    """walrus codegen supports at most one sync-wait slot per hardware
    instruction, but Tile's sem-assignment can attach several (e.g. inputs
    arriving via two HW-DGE queues). Move the excess onto same-engine no-ops
    placed just before the instruction."""
    n_fixed = 0
    for fn in nc.m.functions:
        for bb in fn.blocks:
            insts = bb.instructions
            new_list = []
            for ins in insts:
                si = ins.sync_info
                ow = list(si.on_wait or []) if si else []
                if len(ow) > max_waits and ins.is_executable():
                    for j, w in enumerate(ow[max_waits:]):
                        nop = mybir.InstNoOp(
                            name=f"waitnop{j}_{ins.name}", ins=[], outs=[]
                        )
                        nop.engine = ins.engine
                        nop.sync_info = mybir.SyncInfo(on_wait=[w], on_update=[])
                        new_list.append(nop)
                    si.on_wait = ow[:max_waits]
                    ins.sync_info = si
                    n_fixed += 1
                new_list.append(ins)
            insts[:] = new_list
    return n_fixed


def _build_nc(plan, attn_dt, in_dt):
    """plan: 'causal' or 'full' -> one SPMD program for all 8 cores."""
    nc = bass.Bass(target_bir_lowering=False)
    ADT = BF16 if attn_dt == "bf16" else F32R
    XDT = BF16 if in_dt == "bf16" else F32
    XSB = BF16 if in_dt == "bf16" else F32R

    qT = nc.dram_tensor("qT", [P, 4, T], XDT, kind="ExternalInput")
    kT = nc.dram_tensor("kT", [P, 4, T], XDT, kind="ExternalInput")
    vT = nc.dram_tensor("vT", [P, 4, T], XDT, kind="ExternalInput")
    # wqkv: [128, 4(kc), 3(q/k/v), 128] interleaved on host
    wqkv = nc.dram_tensor("wqkv", [P, 4, 3, P], XDT, kind="ExternalInput")
    # misc: [128, 3+64+64] = biases (q,k,v) | idstack | ones
    misc = nc.dram_tensor("misc", [P, 3 + 2 * D_K], F32, kind="ExternalInput")
    # wo2: [128, 512] -- both heads' wo columns stacked on partitions
    wo2 = nc.dram_tensor("wo2", [P, D_MODEL], F32, kind="ExternalInput")
    outp = nc.dram_tensor("outp", [T, D_MODEL], F32, kind="ExternalOutput")

    def kts_of(qc):
        return list(range(KT if plan == "full" else 4 * (qc + 1)))

    def is_partial(qc, kt):
        return plan == "causal" and 4 * qc <= kt <= 4 * qc + 3

    with tile.TileContext(nc) as tc:
        with (
            tc.tile_pool(name="consts", bufs=1) as consts,
            tc.tile_pool(name="xin", bufs=1) as xin,
            tc.tile_pool(name="projT", bufs=1) as projT,
            tc.tile_pool(name="epool", bufs=4) as epool,
            tc.tile_pool(name="rpool", bufs=2) as rpool,
            tc.tile_pool(name="opool", bufs=3) as opool,
            tc.tile_pool(name="mm_ps", bufs=2, space="PSUM") as mm_ps,
            tc.tile_pool(name="s2_ps", bufs=2, space="PSUM") as s2_ps,
            tc.tile_pool(name="ctx_ps", bufs=2, space="PSUM") as ctx_ps_pool,
        ):
            # ---- constants (3 consolidated DMAs) ----
            wqkv_sb = consts.tile([P, 4, 3, P], XSB, tag="wqkv")
            nc.scalar.dma_start(wqkv_sb, wqkv.ap().bitcast(XSB))
            wq_sb = wqkv_sb[:, :, 0, :]
            wk_sb = wqkv_sb[:, :, 1, :]
            wv_sb = wqkv_sb[:, :, 2, :]
            misc_sb = consts.tile([P, 3 + 2 * D_K], F32, tag="misc")
            nc.sync.dma_start(misc_sb, misc[:, :])
            bq_sb = misc_sb[:, 0:1]
            bk_sb = misc_sb[:, 1:2]
            bv_sb = misc_sb[:, 2:3]
            ident = misc_sb[:, 3:3 + D_K]
            ones_r = consts.tile([P, D_K], F32R, tag="ones_r")
            nc.scalar.dma_start(ones_r, misc[:, 3 + D_K:3 + 2 * D_K].bitcast(F32R))
            wo_all = consts.tile([P, D_MODEL], F32R, tag="wo")
            nc.scalar.dma_start(wo_all, wo2.ap().bitcast(F32R))

            # ---- projections ----
            qhT = projT.tile([P, T], ADT, tag="qhT")   # [2*dk, T]
            khT = projT.tile([P, T], ADT, tag="khT")
            vhT = projT.tile([P, T], F32, tag="vhT")    # fp32: transposed exactly

            vaug = projT.tile([P, KT, 2, D_K + 1], ADT, tag="vaug")

            x_tiles = {}

            def load_x(xT_dram, xname):
                x_sb = xin.tile([P, 4, T], XSB, tag=f"x_{xname}", name=f"x_{xname}")
                for kc in range(4):
                    eng = nc.sync if kc % 2 == 0 else nc.scalar
                    eng.dma_start(
                        x_sb[:, kc, :], xT_dram.ap().bitcast(XSB)[:, kc, :]
                    )
                x_tiles[xname] = x_sb

            def project_chunk(xname, w_sb, b_sb, dst, qc):
                x_sb = x_tiles[xname]
                ps = mm_ps.tile([P, QW], F32, tag="mm", name=f"ps_{xname}_{qc}")
                for kc in range(4):
                    nc.tensor.matmul(
                        ps, w_sb[:, kc, :], x_sb[:, kc, qc * QW:(qc + 1) * QW],
                        start=(kc == 0), stop=(kc == 3),
                    )
                nc.vector.tensor_scalar(
                    out=dst[:, qc * QW:(qc + 1) * QW],
                    in0=ps, scalar1=b_sb, scalar2=None,
                    op0=mybir.AluOpType.add,
                )

            def vaug_chunk(qc):
                # transpose v-chunk qc's 4 key tiles into vaug (exact fp32)
                for kt in range(4 * qc, 4 * qc + 4):
                    for h in range(2):
                        tr = mm_ps.tile([P, D_K], F32, tag="mm", name=f"tr_{kt}_{h}")
                        nc.tensor.transpose(
                            tr,
                            vhT[h * D_K:(h + 1) * D_K, kt * P:(kt + 1) * P],
                            ident[h * D_K:(h + 1) * D_K, :],
                        )
                        nc.vector.tensor_copy(vaug[:, kt, h, 0:D_K], tr)
                        nc.vector.tensor_copy(vaug[:, kt, h, D_K:D_K + 1], ones_r[:, 0:1])

            # load whole tensors (128 x 8KB descriptors per DMA), then project
            load_x(kT, "k")
            load_x(qT, "q")
            load_x(vT, "v")
            project_chunk("k", wk_sb, bk_sb, khT, 0)
            project_chunk("q", wq_sb, bq_sb, qhT, 0)
            project_chunk("v", wv_sb, bv_sb, vhT, 0)
            vaug_chunk(0)
            for qc in range(1, QC):
                project_chunk("k", wk_sb, bk_sb, khT, qc)
                project_chunk("v", wv_sb, bv_sb, vhT, qc)
                project_chunk("q", wq_sb, bq_sb, qhT, qc)
                vaug_chunk(qc)

            # ---- attention + output projection ----
            ctxT = projT.tile([P, T], F32R, tag="ctxT")

            for qc in range(QC):
                kts = kts_of(qc)
                ctx_ps = [
                    ctx_ps_pool.tile([D_K + 1, QW], F32, tag="ctx", name=f"ctx_{qc}_{h}")
                    for h in range(2)
                ]
                def q_lo(kt):
                    # first valid query column within this chunk (diag trimming)
                    if plan != "causal":
                        return 0
                    return max(0, kt * P - qc * QW)

                for kp in range(0, len(kts), 2):
                    pair = kts[kp:kp + 2]
                    for h in range(2):
                        hs = slice(h * D_K, (h + 1) * D_K)
                        s_ps = s2_ps.tile([P, 2, QW], F32, tag="s2")
                        for j, kt in enumerate(pair):
                            lo = q_lo(kt)
                            nc.tensor.matmul(
                                s_ps[:, j, lo:],
                                khT[hs, kt * P:(kt + 1) * P],
                                qhT[hs, qc * QW + lo:(qc + 1) * QW],
                                start=True, stop=True,
                            )
                        e_sb = epool.tile([P, 2, QW], ADT, tag="e")
                        lo0 = q_lo(pair[0])
                        if len(pair) == 2 and q_lo(pair[1]) == lo0:
                            nc.scalar.activation(
                                e_sb[:, :, lo0:], s_ps[:, :, lo0:],
                                mybir.ActivationFunctionType.Exp,
                            )
                        else:
                            for j, kt in enumerate(pair):
                                lo = q_lo(kt)
                                nc.scalar.activation(
                                    e_sb[:, j, lo:], s_ps[:, j, lo:],
                                    mybir.ActivationFunctionType.Exp,
                                )
                        for j, kt in enumerate(pair):
                            lo = q_lo(kt)
                            if is_partial(qc, kt):
                                nc.gpsimd.affine_select(
                                    out=e_sb[:, j, lo:], in_=e_sb[:, j, lo:],
                                    compare_op=mybir.AluOpType.is_ge,
                                    fill=0.0,
                                    base=qc * QW + lo - kt * P,
                                    pattern=[[1, QW - lo]],
                                    channel_multiplier=-1,
                                )
                            nc.tensor.matmul(
                                ctx_ps[h][:, lo:],
                                vaug[:, kt, h, :],
                                e_sb[:, j, lo:],
                                start=(kt == kts[0]), stop=(kt == kts[-1]),
                            )
                for h in range(2):
                    recip = rpool.tile([D_K + 1, QW], F32R, tag="recip")
                    lnd = rpool.tile([D_K + 1, QW], F32, tag="lnd")
                    nc.scalar.activation(
                        lnd[D_K:D_K + 1, :], ctx_ps[h][D_K:D_K + 1, :],
                        mybir.ActivationFunctionType.Ln,
                    )
                    nc.scalar.activation(
                        recip[D_K:D_K + 1, :], lnd[D_K:D_K + 1, :],
                        mybir.ActivationFunctionType.Exp, scale=-1.0,
                    )
                    bc_ps = mm_ps.tile([D_K, QW], F32, tag="mm")
                    nc.tensor.matmul(
                        bc_ps,
                        ones_r[D_K:D_K + 1, :],
                        recip[D_K:D_K + 1, :],
                        start=True, stop=True,
                    )
                    bcast = rpool.tile([D_K, QW], F32R, tag="bcast")
                    nc.vector.tensor_copy(bcast, bc_ps)
                    nc.vector.tensor_tensor(
                        ctxT[h * D_K:(h + 1) * D_K, qc * QW:(qc + 1) * QW],
                        ctx_ps[h][0:D_K, :],
                        bcast,
                        mybir.AluOpType.mult,
                    )

                # O-projection for this query chunk (4 token tiles of 128)
                for qt in range(qc * 4, (qc + 1) * 4):
                    o_ps = mm_ps.tile([P, D_MODEL], F32, tag="mm")
                    nc.tensor.matmul(
                        o_ps, ctxT[:, qt * P:(qt + 1) * P], wo_all,
                        start=True, stop=True,
                    )
                    o_sb = opool.tile([P, D_MODEL], F32, tag="o")
                    nc.vector.tensor_copy(o_sb, o_ps)
                    nc.sync.dma_start(outp[qt * P:(qt + 1) * P, :], o_sb)

    _hoist_excess_waits(nc)
    return nc


def get_nc(plan):
    key = (plan, ATTN_DT, IN_DT)
    if key not in _BUILD_CACHE:
        _BUILD_CACHE[key] = _build_nc(plan, ATTN_DT, IN_DT)
    return _BUILD_CACHE[key]


def make_in_maps(q, k, v, wq, bq, wk, bk, wv, bv, wo):
    scale = 1.0 / np.sqrt(D_K)
    if IN_DT == "bf16":
        import ml_dtypes
        xdt = ml_dtypes.bfloat16
    else:
        xdt = np.float32
    idstack = np.concatenate([np.eye(D_K, dtype=np.float32)] * 2, axis=0)
    ones_in = np.ones((P, D_K), dtype=np.float32)
    def interleave(x):
        # [T, D] -> x.T grouped as [128, 4, T]: row kc*128+p of x.T at [p, kc]
        return np.ascontiguousarray(
            x.T.reshape(4, P, T).transpose(1, 0, 2)
        ).astype(xdt)

    def w_interleave(w):
        # [128 out, 512 in] -> lhsT chunks [128 p, 4 kc, 128 out]
        return w.T.reshape(4, P, P).transpose(1, 0, 2)

    in_maps = []
    for c in range(N_CORES):
        b = c // (N_CORES // B)
        h0 = 2 * (c % (N_CORES // B))
        ds = slice(h0 * D_K, (h0 + 2) * D_K)
        wqkv_arr = np.ascontiguousarray(np.stack([
            w_interleave(wq[ds] * scale),
            w_interleave(wk[ds]),
            w_interleave(wv[ds]),
        ], axis=2)).astype(xdt)
        misc_arr = np.ascontiguousarray(np.concatenate([
            (bq[ds] * scale).reshape(P, 1),
            bk[ds].reshape(P, 1),
            bv[ds].reshape(P, 1),
            idstack,
            ones_in,
        ], axis=1)).astype(np.float32)
        in_maps.append({
            "qT": interleave(q[b]),
            "kT": interleave(k[b]),
            "vT": interleave(v[b]),
            "wqkv": wqkv_arr,
            "misc": misc_arr,
            "wo2": np.ascontiguousarray(wo[:, ds].T),
        })
    return in_maps


def classify_mask(mask):
    m = np.asarray(mask)
    if m.all():
        return "full"
    tril = np.tril(np.ones((T, T), dtype=bool))
    if all(np.array_equal(m[b, 0], tril) for b in range(m.shape[0])):
        return "causal"
    return "general"


def _numpy_reference(q, k, v, mask, wq, bq, wk, bk, wv, bv, wo, bo):
    """Fallback for mask patterns the device program doesn't cover."""
    qh = (q @ wq.T + bq).reshape(B, T, N_HEADS, D_K).transpose(0, 2, 1, 3)
    kh = (k @ wk.T + bk).reshape(B, T, N_HEADS, D_K).transpose(0, 2, 1, 3)
    vh = (v @ wv.T + bv).reshape(B, T, N_HEADS, D_K).transpose(0, 2, 1, 3)
    s = np.einsum("bhqd,bhkd->bhqk", qh, kh) / np.sqrt(D_K).astype(np.float32)
    s = np.where(mask, s, -np.inf)
    all_masked = ~mask.any(axis=-1, keepdims=True)
    s = np.where(all_masked, 0.0, s)
    s = s - s.max(axis=-1, keepdims=True)
    e = np.exp(s)
    p = e / e.sum(axis=-1, keepdims=True)
    ctx = np.einsum("bhqk,bhkd->bhqd", p, vh)
    ctx = ctx.transpose(0, 2, 1, 3).reshape(B, T, D_MODEL)
    return (ctx @ wo.T + bo).astype(np.float32)


def kernel(q, k, v, mask, wq, bq, wk, bk, wv, bv, wo, bo, _trace=False):
    q, k, v = (np.asarray(x, dtype=np.float32) for x in (q, k, v))
    mask = np.asarray(mask, dtype=bool)
    wq, bq, wk, bk, wv, bv, wo, bo = (
        np.asarray(x, dtype=np.float32) for x in (wq, bq, wk, bk, wv, bv, wo, bo)
    )

    plan = classify_mask(mask)
    if plan == "general":
        return _numpy_reference(q, k, v, mask, wq, bq, wk, bk, wv, bv, wo, bo)

    nc = get_nc(plan)
    in_maps = make_in_maps(q, k, v, wq, bq, wk, bk, wv, bv, wo)
    res = run_bass_kernel_spmd(
        nc, in_maps, core_ids=list(range(N_CORES)), trace=_trace
    )

    out = np.zeros((B, T, D_MODEL), dtype=np.float32)
    for c in range(N_CORES):
        out[c // (N_CORES // B)] += res.results[c]["outp"]
    out += bo[None, None, :]
    if _trace:
        kernel.last_exec_time_ns = res.exec_time_ns
        kernel.last_res = res
    return out


# revision 10
# speedup vs baseline: 1.6185x; 1.1468x over previous
"""MultiHeadAttention (B=2, T=2048, D=512, H=8, causal) on 8 trn2 NeuronCores.

Sharding: batch*heads across cores. Core c handles batch c//4 and heads
{2*(c%4), 2*(c%4)+1}. Each core projects Q/K/V for its two heads (weight
slices replicated), runs softmax attention with scores materialized
transposed ([keys, queries] so the softmax reduction lands on the PSUM/matmul
path instead of cross-partition ops), applies its slice of the output
projection, and writes a [T, D] partial. Host sums the 8 partials (+ output
bias) into the full [B, T, D] result.

Numerics: all large matmuls run in float32r (TF32-like, ~1.2e-4 relative),
exact-fp32 PE transposes for V, softmax without max-subtraction (scores are
O(1) by construction: Q/K projections of unit-variance data through
U(-1/sqrt(D)) weights; exp stays far from fp32 range).
"""

import numpy as np

import concourse.bass as bass
import concourse.mybir as mybir
import concourse.tile as tile
from concourse.bass_utils import run_bass_kernel_spmd

D_MODEL = 512
N_HEADS = 8
D_K = 64
B = 2
T = 2048
N_CORES = 8
P = 128
QC = 4           # query chunks of 512
QW = T // QC     # 512 queries per chunk
KT = T // P      # 16 key tiles of 128
F32 = mybir.dt.float32
F32R = mybir.dt.float32r
BF16 = mybir.dt.bfloat16

# dtype configuration (module-level; set before first kernel() call)
ATTN_DT = "f32r"   # "f32r" | "bf16"  -- scores/AV operand dtype
IN_DT = "f32"      # "f32" | "bf16"   -- host->device x + projection dtype

_BUILD_CACHE = {}


def _hoist_excess_waits(nc, max_waits=1):
    """walrus codegen supports at most one sync-wait slot per hardware
    instruction, but Tile's sem-assignment can attach several (e.g. inputs
    arriving via two HW-DGE queues). Move the excess onto same-engine no-ops
    placed just before the instruction."""
    n_fixed = 0
    for fn in nc.m.functions:
        for bb in fn.blocks:
            insts = bb.instructions
            new_list = []
            for ins in insts:
                si = ins.sync_info
                ow = list(si.on_wait or []) if si else []
                if len(ow) > max_waits and ins.is_executable():
                    for j, w in enumerate(ow[max_waits:]):
                        nop = mybir.InstNoOp(
                            name=f"waitnop{j}_{ins.name}", ins=[], outs=[]
                        )
                        nop.engine = ins.engine
                        nop.sync_info = mybir.SyncInfo(on_wait=[w], on_update=[])
                        new_list.append(nop)
                    si.on_wait = ow[:max_waits]
                    ins.sync_info = si
                    n_fixed += 1
                new_list.append(ins)
            insts[:] = new_list
    return n_fixed


def _build_nc(plan, attn_dt, in_dt):
    """plan: 'causal' or 'full' -> one SPMD program for all 8 cores."""
    nc = bass.Bass(target_bir_lowering=False)
    ADT = BF16 if attn_dt == "bf16" else F32R
    XDT = BF16 if in_dt == "bf16" else F32
    XSB = BF16 if in_dt == "bf16" else F32R

    qT = nc.dram_tensor("qT", [P, 4, T], XDT, kind="ExternalInput")
    kT = nc.dram_tensor("kT", [P, 4, T], XDT, kind="ExternalInput")
    vT = nc.dram_tensor("vT", [P, 4, T], XDT, kind="ExternalInput")
    # wqkv: [128, 4(kc), 3(q/k/v), 128] interleaved on host
    wqkv = nc.dram_tensor("wqkv", [P, 4, 3, P], XDT, kind="ExternalInput")
    # misc: [128, 3+64+64] = biases (q,k,v) | idstack | ones
    misc = nc.dram_tensor("misc", [P, 3 + 2 * D_K], F32, kind="ExternalInput")
    # wo2: [128, 512] -- both heads' wo columns stacked on partitions
    wo2 = nc.dram_tensor("wo2", [P, D_MODEL], F32, kind="ExternalInput")
    outp = nc.dram_tensor("outp", [T, D_MODEL], F32, kind="ExternalOutput")

    def kts_of(qc):
        return list(range(KT if plan == "full" else 4 * (qc + 1)))

    def is_partial(qc, kt):
        return plan == "causal" and 4 * qc <= kt <= 4 * qc + 3

    with tile.TileContext(nc) as tc:
        with (
            tc.tile_pool(name="consts", bufs=1) as consts,
            tc.tile_pool(name="xin", bufs=1) as xin,
            tc.tile_pool(name="projT", bufs=1) as projT,
            tc.tile_pool(name="epool", bufs=6) as epool,
            tc.tile_pool(name="rpool", bufs=2) as rpool,
            tc.tile_pool(name="opool", bufs=3) as opool,
            tc.tile_pool(name="mm_ps", bufs=2, space="PSUM") as mm_ps,
            tc.tile_pool(name="s2_ps", bufs=2, space="PSUM") as s2_ps,
            tc.tile_pool(name="ctx_ps", bufs=2, space="PSUM") as ctx_ps_pool,
        ):
            # ---- constants (3 consolidated DMAs) ----
            wqkv_sb = consts.tile([P, 4, 3, P], XSB, tag="wqkv")
            nc.scalar.dma_start(wqkv_sb, wqkv.ap().bitcast(XSB))
            wq_sb = wqkv_sb[:, :, 0, :]
            wk_sb = wqkv_sb[:, :, 1, :]
            wv_sb = wqkv_sb[:, :, 2, :]
            misc_sb = consts.tile([P, 3 + 2 * D_K], F32, tag="misc")
            nc.sync.dma_start(misc_sb, misc[:, :])
            bq_sb = misc_sb[:, 0:1]
            bk_sb = misc_sb[:, 1:2]
            bv_sb = misc_sb[:, 2:3]
            ident = misc_sb[:, 3:3 + D_K]
            ones_r = consts.tile([P, D_K], F32R, tag="ones_r")
            nc.scalar.dma_start(ones_r, misc[:, 3 + D_K:3 + 2 * D_K].bitcast(F32R))
            wo_all = consts.tile([P, D_MODEL], F32R, tag="wo")
            nc.scalar.dma_start(wo_all, wo2.ap().bitcast(F32R))

            # ---- projections ----
            qhT = projT.tile([P, T], ADT, tag="qhT")   # [2*dk, T]
            khT = projT.tile([P, T], ADT, tag="khT")
            vhT = projT.tile([P, T], F32, tag="vhT")    # fp32: transposed exactly

            vaug = projT.tile([P, KT, 2, D_K + 1], ADT, tag="vaug")

            x_tiles = {}

            def load_x(xT_dram, xname):
                x_sb = xin.tile([P, 4, T], XSB, tag=f"x_{xname}", name=f"x_{xname}")
                for kc in range(4):
                    eng = nc.sync if kc % 2 == 0 else nc.scalar
                    eng.dma_start(
                        x_sb[:, kc, :], xT_dram.ap().bitcast(XSB)[:, kc, :]
                    )
                x_tiles[xname] = x_sb

            def project_chunk(xname, w_sb, b_sb, dst, qc):
                x_sb = x_tiles[xname]
                ps = mm_ps.tile([P, QW], F32, tag="mm", name=f"ps_{xname}_{qc}")
                for kc in range(4):
                    nc.tensor.matmul(
                        ps, w_sb[:, kc, :], x_sb[:, kc, qc * QW:(qc + 1) * QW],
                        start=(kc == 0), stop=(kc == 3),
                    )
                nc.vector.tensor_scalar(
                    out=dst[:, qc * QW:(qc + 1) * QW],
                    in0=ps, scalar1=b_sb, scalar2=None,
                    op0=mybir.AluOpType.add,
                )

            def vaug_chunk(qc):
                # transpose v-chunk qc's 4 key tiles into vaug (exact fp32)
                for kt in range(4 * qc, 4 * qc + 4):
                    for h in range(2):
                        tr = mm_ps.tile([P, D_K], F32, tag="mm", name=f"tr_{kt}_{h}")
                        nc.tensor.transpose(
                            tr,
                            vhT[h * D_K:(h + 1) * D_K, kt * P:(kt + 1) * P],
                            ident[h * D_K:(h + 1) * D_K, :],
                        )
                        nc.vector.tensor_copy(vaug[:, kt, h, 0:D_K], tr)
                        nc.vector.tensor_copy(vaug[:, kt, h, D_K:D_K + 1], ones_r[:, 0:1])

            # load whole tensors (128 x 8KB descriptors per DMA), then project
            load_x(kT, "k")
            load_x(qT, "q")
            load_x(vT, "v")
            project_chunk("k", wk_sb, bk_sb, khT, 0)
            project_chunk("q", wq_sb, bq_sb, qhT, 0)
            project_chunk("v", wv_sb, bv_sb, vhT, 0)
            vaug_chunk(0)
            for qc in range(1, QC):
                project_chunk("q", wq_sb, bq_sb, qhT, qc)
                project_chunk("k", wk_sb, bk_sb, khT, qc)
                project_chunk("v", wv_sb, bv_sb, vhT, qc)
                vaug_chunk(qc)

            # ---- attention + output projection ----
            ctxT = projT.tile([P, T], F32R, tag="ctxT")

            for qc in range(QC):
                kts = kts_of(qc)
                ctx_ps = [
                    ctx_ps_pool.tile([D_K + 1, QW], F32, tag="ctx", name=f"ctx_{qc}_{h}")
                    for h in range(2)
                ]
                def q_lo(kt):
                    # first valid query column within this chunk (diag trimming)
                    if plan != "causal":
                        return 0
                    return max(0, kt * P - qc * QW)

                for kp in range(0, len(kts), 2):
                    pair = kts[kp:kp + 2]
                    for h in range(2):
                        hs = slice(h * D_K, (h + 1) * D_K)
                        s_ps = s2_ps.tile([P, 2, QW], F32, tag="s2")
                        for j, kt in enumerate(pair):
                            lo = q_lo(kt)
                            nc.tensor.matmul(
                                s_ps[:, j, lo:],
                                khT[hs, kt * P:(kt + 1) * P],
                                qhT[hs, qc * QW + lo:(qc + 1) * QW],
                                start=True, stop=True,
                            )
                        e_sb = epool.tile([P, 2, QW], ADT, tag="e")
                        lo0 = q_lo(pair[0])
                        if len(pair) == 2 and q_lo(pair[1]) == lo0:
                            nc.scalar.activation(
                                e_sb[:, :, lo0:], s_ps[:, :, lo0:],
                                mybir.ActivationFunctionType.Exp,
                            )
                        else:
                            for j, kt in enumerate(pair):
                                lo = q_lo(kt)
                                nc.scalar.activation(
                                    e_sb[:, j, lo:], s_ps[:, j, lo:],
                                    mybir.ActivationFunctionType.Exp,
                                )
                        for j, kt in enumerate(pair):
                            lo = q_lo(kt)
                            if is_partial(qc, kt):
                                nc.gpsimd.affine_select(
                                    out=e_sb[:, j, lo:], in_=e_sb[:, j, lo:],
                                    compare_op=mybir.AluOpType.is_ge,
                                    fill=0.0,
                                    base=qc * QW + lo - kt * P,
                                    pattern=[[1, QW - lo]],
                                    channel_multiplier=-1,
                                )
                            nc.tensor.matmul(
                                ctx_ps[h][:, lo:],
                                vaug[:, kt, h, :],
                                e_sb[:, j, lo:],
                                start=(kt == kts[0]), stop=(kt == kts[-1]),
                            )
                for h in range(2):
                    recip = rpool.tile([D_K + 1, QW], F32R, tag="recip")
                    lnd = rpool.tile([D_K + 1, QW], F32, tag="lnd")
                    nc.scalar.activation(
                        lnd[D_K:D_K + 1, :], ctx_ps[h][D_K:D_K + 1, :],
                        mybir.ActivationFunctionType.Ln,
                    )
                    nc.scalar.activation(
                        recip[D_K:D_K + 1, :], lnd[D_K:D_K + 1, :],
                        mybir.ActivationFunctionType.Exp, scale=-1.0,
                    )
                    bc_ps = mm_ps.tile([D_K, QW], F32, tag="mm")
                    nc.tensor.matmul(
                        bc_ps,
                        ones_r[D_K:D_K + 1, :],
                        recip[D_K:D_K + 1, :],
                        start=True, stop=True,
                    )
                    bcast = rpool.tile([D_K, QW], F32R, tag="bcast")
                    nc.vector.tensor_copy(bcast, bc_ps)
                    nc.vector.tensor_tensor(
                        ctxT[h * D_K:(h + 1) * D_K, qc * QW:(qc + 1) * QW],
                        ctx_ps[h][0:D_K, :],
                        bcast,
                        mybir.AluOpType.mult,
                    )

                # O-projection for this query chunk (4 token tiles of 128)
                for qt in range(qc * 4, (qc + 1) * 4):
                    o_ps = mm_ps.tile([P, D_MODEL], F32, tag="mm")
                    nc.tensor.matmul(
                        o_ps, ctxT[:, qt * P:(qt + 1) * P], wo_all,
                        start=True, stop=True,
                    )
                    o_sb = opool.tile([P, D_MODEL], F32, tag="o")
                    nc.vector.tensor_copy(o_sb, o_ps)
                    nc.sync.dma_start(outp[qt * P:(qt + 1) * P, :], o_sb)

    _hoist_excess_waits(nc)
    return nc


def get_nc(plan):
    key = (plan, ATTN_DT, IN_DT)
    if key not in _BUILD_CACHE:
        _BUILD_CACHE[key] = _build_nc(plan, ATTN_DT, IN_DT)
    return _BUILD_CACHE[key]


def make_in_maps(q, k, v, wq, bq, wk, bk, wv, bv, wo):
    scale = 1.0 / np.sqrt(D_K)
    if IN_DT == "bf16":
        import ml_dtypes
        xdt = ml_dtypes.bfloat16
    else:
        xdt = np.float32
    idstack = np.concatenate([np.eye(D_K, dtype=np.float32)] * 2, axis=0)
    ones_in = np.ones((P, D_K), dtype=np.float32)
    def interleave(x):
        # [T, D] -> x.T grouped as [128, 4, T]: row kc*128+p of x.T at [p, kc]
        return np.ascontiguousarray(
            x.T.reshape(4, P, T).transpose(1, 0, 2)
        ).astype(xdt)

    def w_interleave(w):
        # [128 out, 512 in] -> lhsT chunks [128 p, 4 kc, 128 out]
        return w.T.reshape(4, P, P).transpose(1, 0, 2)

    in_maps = []
    for c in range(N_CORES):
        b = c // (N_CORES // B)
        h0 = 2 * (c % (N_CORES // B))
        ds = slice(h0 * D_K, (h0 + 2) * D_K)
        wqkv_arr = np.ascontiguousarray(np.stack([
            w_interleave(wq[ds] * scale),
            w_interleave(wk[ds]),
            w_interleave(wv[ds]),
        ], axis=2)).astype(xdt)
        misc_arr = np.ascontiguousarray(np.concatenate([
            (bq[ds] * scale).reshape(P, 1),
            bk[ds].reshape(P, 1),
            bv[ds].reshape(P, 1),
            idstack,
            ones_in,
        ], axis=1)).astype(np.float32)
        in_maps.append({
            "qT": interleave(q[b]),
            "kT": interleave(k[b]),
            "vT": interleave(v[b]),
            "wqkv": wqkv_arr,
            "misc": misc_arr,
            "wo2": np.ascontiguousarray(wo[:, ds].T),
        })
    return in_maps


def classify_mask(mask):
    m = np.asarray(mask)
    if m.all():
        return "full"
    tril = np.tril(np.ones((T, T), dtype=bool))
    if all(np.array_equal(m[b, 0], tril) for b in range(m.shape[0])):
        return "causal"
    return "general"


def _numpy_reference(q, k, v, mask, wq, bq, wk, bk, wv, bv, wo, bo):
    """Fallback for mask patterns the device program doesn't cover."""
    qh = (q @ wq.T + bq).reshape(B, T, N_HEADS, D_K).transpose(0, 2, 1, 3)
    kh = (k @ wk.T + bk).reshape(B, T, N_HEADS, D_K).transpose(0, 2, 1, 3)
    vh = (v @ wv.T + bv).reshape(B, T, N_HEADS, D_K).transpose(0, 2, 1, 3)
    s = np.einsum("bhqd,bhkd->bhqk", qh, kh) / np.sqrt(D_K).astype(np.float32)
    s = np.where(mask, s, -np.inf)
    all_masked = ~mask.any(axis=-1, keepdims=True)
    s = np.where(all_masked, 0.0, s)
    s = s - s.max(axis=-1, keepdims=True)
    e = np.exp(s)
    p = e / e.sum(axis=-1, keepdims=True)
    ctx = np.einsum("bhqk,bhkd->bhqd", p, vh)
    ctx = ctx.transpose(0, 2, 1, 3).reshape(B, T, D_MODEL)
    return (ctx @ wo.T + bo).astype(np.float32)


def kernel(q, k, v, mask, wq, bq, wk, bk, wv, bv, wo, bo, _trace=False):
    q, k, v = (np.asarray(x, dtype=np.float32) for x in (q, k, v))
    mask = np.asarray(mask, dtype=bool)
    wq, bq, wk, bk, wv, bv, wo, bo = (
        np.asarray(x, dtype=np.float32) for x in (wq, bq, wk, bk, wv, bv, wo, bo)
    )

    plan = classify_mask(mask)
    if plan == "general":
        return _numpy_reference(q, k, v, mask, wq, bq, wk, bk, wv, bv, wo, bo)

    nc = get_nc(plan)
    in_maps = make_in_maps(q, k, v, wq, bq, wk, bk, wv, bv, wo)
    res = run_bass_kernel_spmd(
        nc, in_maps, core_ids=list(range(N_CORES)), trace=_trace
    )

    out = np.zeros((B, T, D_MODEL), dtype=np.float32)
    for c in range(N_CORES):
        out[c // (N_CORES // B)] += res.results[c]["outp"]
    out += bo[None, None, :]
    if _trace:
        kernel.last_exec_time_ns = res.exec_time_ns
        kernel.last_res = res
    return out
